# revision 1
# baseline (speedup 1.0000x reference)
"""Trainium2 Bass kernel for DecomposableAttention (B=2, L=4096, E=1024, H=2048, O=3).

Sharding: 8 cores = 2 groups of 4 (one per batch element). Within a group the
sequence dim L is sharded 4 ways (Ls=1024 rows per core). Cross-core data:
  - AllGather of the f-MLP outputs faT/fbT (needed for the full attention bmm)
  - ReduceScatter of exp-row/col-sum partials (softmax denominators)
  - ReduceScatter / AllReduce for the tiny aggregate h-MLP.
Both attention orientations exp(fa@fb^T) and exp(fb@fa^T) are computed locally
([all x shard] each) so that beta and alpha contractions are fully local.

All matmuls run as float32r (FP22 truncate, fp32 accumulate) at full PE rate.
"""

import numpy as np

try:
    import concourse.bass as bass
except ImportError:  # fall back to the staged repo checkout
    import sys
    for p in ("/opt/trn_rl_repo", "/root/.axon_site/_ro/trn_rl_repo"):
        if p not in sys.path:
            sys.path.insert(0, p)
    import concourse.bass as bass
import concourse.mybir as mybir
import concourse.tile as tile
from concourse import bacc
from concourse import bass_utils

F32 = mybir.dt.float32
F32R = mybir.dt.float32r
AF = mybir.ActivationFunctionType
ALU = mybir.AluOpType
P = 128
CH = 512  # moving free-dim chunk (1 fp32 PSUM bank)


def _r(ap):
    return ap.bitcast(F32R)


def build_nc(B=2, L=4096, E=1024, H=2048, O=3, n_cores=8, reps=1,
             mock_collectives=False, phases=None):
    """Build the SPMD Bass program (identical on all cores; per-core inputs)."""
    G = n_cores // B          # cores per batch group
    Ls = L // G               # sequence shard per core
    ET, HT, IT = E // P, H // P, L // P       # 128-tiles per dim
    CHN = Ls // CH            # free chunks per token block
    KT1 = 2 * H // P          # agg dim tiles (h layer 1)
    KS1 = KT1 // G            # per-core k-tiles for hW1
    KS2 = (H // P) // G       # per-core k-tiles for hW2
    assert Ls % CH == 0 and E % P == 0 and H % P == 0 and (2 * H) % (G * P) == 0
    assert (H // P) % G == 0

    groups = [list(range(g * G, (g + 1) * G)) for g in range(B)]

    nc = bacc.Bacc("TRN2", target_bir_lowering=False, debug=False,
                   num_devices=n_cores)

    # ---------------- external I/O ----------------
    xaT = nc.dram_tensor("xaT", [E, Ls], F32, kind="ExternalInput")
    xbT = nc.dram_tensor("xbT", [E, Ls], F32, kind="ExternalInput")
    x1f = nc.dram_tensor("x1f", [L, E], F32, kind="ExternalInput")
    x2f = nc.dram_tensor("x2f", [L, E], F32, kind="ExternalInput")
    w_in = {}
    for nm in ("f", "g"):
        w_in[nm + "W1"] = nc.dram_tensor(nm + "W1", [E, H], F32, kind="ExternalInput")
        w_in[nm + "W2"] = nc.dram_tensor(nm + "W2", [H, H], F32, kind="ExternalInput")
        w_in[nm + "W3"] = nc.dram_tensor(nm + "W3", [H, H], F32, kind="ExternalInput")
        for i in (1, 2, 3):
            w_in[f"{nm}b{i}"] = nc.dram_tensor(f"{nm}b{i}", [H], F32, kind="ExternalInput")
    hW1s = nc.dram_tensor("hW1s", [2 * H // G, H], F32, kind="ExternalInput")
    hW2s = nc.dram_tensor("hW2s", [H // G, H], F32, kind="ExternalInput")
    hW3 = nc.dram_tensor("hW3", [H, O], F32, kind="ExternalInput")
    hb1s = nc.dram_tensor("hb1s", [H // G], F32, kind="ExternalInput")
    hb2 = nc.dram_tensor("hb2", [H], F32, kind="ExternalInput")
    hb3 = nc.dram_tensor("hb3", [O], F32, kind="ExternalInput")
    out = nc.dram_tensor("out", [O, 1], F32, kind="ExternalOutput")

    with tile.TileContext(nc) as tc:
        with (
            tc.tile_pool(name="big", bufs=1) as bigp,
            tc.tile_pool(name="med", bufs=1) as medp,
            tc.tile_pool(name="wst", bufs=2) as wstp,
            tc.tile_pool(name="row", bufs=3) as rowp,
            tc.tile_pool(name="ev", bufs=2) as evp,
            tc.tile_pool(name="sml", bufs=1) as smlp,
            tc.tile_pool(name="ps", bufs=8, space="PSUM") as psp,
            tc.tile_pool(name="dram", bufs=1, space="DRAM") as dramp,
        ):
            on = lambda p: phases is None or p in phases
            for _rep in range(reps):
                def cc(kind, op, replica_groups, tin, tout):
                    if not mock_collectives:
                        nc.gpsimd.collective_compute(kind, op,
                                                     replica_groups=replica_groups,
                                                     ins=[tin.opt()],
                                                     outs=[tout.opt()])
                        return
                    if kind == "AllGather":
                        for s in range(G):
                            nc.sync.dma_start(tout[s], tin[:])
                    elif kind == "ReduceScatter":
                        if len(tin.shape) > 1 and tin.shape[0] == G:
                            nc.sync.dma_start(tout[:], tin[0])
                        else:
                            nc.sync.dma_start(tout[:], tin[:tout.shape[0]])
                    else:  # AllReduce
                        nc.sync.dma_start(tout[:], tin[:])

                def dma_split(dst_ap, src_ap, n):
                    K = dst_ap.shape[1]
                    step = max(1, (K + n - 1) // n)
                    for s in range(0, K, step):
                        e = min(K, s + step)
                        nc.sync.dma_start(dst_ap[:, s:e], src_ap[:, s:e])

                # ---------------- DRAM scratch ----------------
                ag_in = dramp.tile([2, H, Ls], F32)            # faT_r, fbT_r
                ag_out = dramp.tile([G, 2, H, Ls], F32)        # gathered faT/fbT
                tA = dramp.tile([IT, P, Ls], F32)              # exp(S)  [all i, my j]
                tB = dramp.tile([IT, P, Ls], F32)              # exp(S^T)[all j, my i]
                rc_in = dramp.tile([G, 2, Ls], F32)            # r/c partial sums
                rc_out = dramp.tile([2, Ls], F32)              # RS result (my shard)
                sp_beta = dramp.tile([ET, P, Ls], F32)         # betaT spill
                sp_alpha = dramp.tile([ET, P, Ls], F32)
                vs_in = dramp.tile([2 * H], F32)               # agg partial
                vs_out = dramp.tile([2 * H // G], F32)         # RS: my agg k-slice
                h1_in = dramp.tile([H], F32)
                h1_out = dramp.tile([H // G], F32)
                h2_in = dramp.tile([H], F32)
                h2_out = dramp.tile([H], F32)

                # ---------------- bias tiles ----------------
                btiles = {}
                for nm in ("fb1", "fb2", "fb3", "gb1", "gb2", "gb3"):
                    t = smlp.tile([P, HT], F32, name=f"bt_{nm}", tag=f"bt_{nm}")
                    nc.sync.dma_start(t[:], w_in[nm].rearrange("(m p) -> p m", p=P))
                    btiles[nm] = t
                hb1s_t = smlp.tile([P, KS2], F32, tag="hb1s_t")
                nc.sync.dma_start(hb1s_t[:], hb1s.rearrange("(m p) -> p m", p=P))
                hb2_t = smlp.tile([P, HT], F32, tag="hb2_t")
                nc.sync.dma_start(hb2_t[:], hb2.rearrange("(m p) -> p m", p=P))
                hb3_t = smlp.tile([O, 1], F32, tag="hb3_t")
                nc.sync.dma_start(hb3_t[:], hb3[:, None])

                # ---------------- helpers ----------------
                def mlp3(src_ap, kt_in, W1, W2, W3, b1, b2, b3, consume, pfx):
                    """3-layer MLP (feature-major activations [P, kt, Ls]), ReLU each
                    layer. src_ap: DRAM AP [P, kt_in, Ls]. consume(m, ch, psum) is
                    called for the layer-3 output instead of materializing it."""
                    in_t = medp.tile([P, ET, Ls], F32, name=f"{pfx}_in", tag="inacts")
                    dma_split(in_t[:, :kt_in, :].bitcast(F32R),
                              src_ap.bitcast(F32R), 4)
                    h1 = bigp.tile([P, HT, Ls], F32, name=f"{pfx}_h1", tag="bigA")
                    for m in range(HT):
                        ws = wstp.tile([P, HT, P], F32, name=f"{pfx}_w1", tag="wst")
                        nc.sync.dma_start(
                            ws[:, :kt_in, :].bitcast(F32R),
                            W1[:, m * P:(m + 1) * P]
                            .rearrange("(k p) m -> p k m", p=P).bitcast(F32R))
                        for ch in range(CHN):
                            ps = psp.tile([P, CH], F32, name=f"{pfx}_ps1", tag="ps")
                            for k in range(kt_in):
                                nc.tensor.matmul(
                                    ps[:], _r(ws[:, k, :]),
                                    _r(in_t[:, k, ch * CH:(ch + 1) * CH]),
                                    start=(k == 0), stop=(k == kt_in - 1))
                            nc.scalar.activation(
                                h1[:, m, ch * CH:(ch + 1) * CH].bitcast(F32R), ps[:],
                                                 AF.Relu, bias=b1[:, m:m + 1])
                    h2 = bigp.tile([P, HT, Ls], F32, name=f"{pfx}_h2", tag="bigB")
                    for m in range(HT):
                        ws = wstp.tile([P, HT, P], F32, name=f"{pfx}_w2", tag="wst")
                        nc.sync.dma_start(
                            ws[:].bitcast(F32R),
                            W2[:, m * P:(m + 1) * P]
                            .rearrange("(k p) m -> p k m", p=P).bitcast(F32R))
                        for ch in range(CHN):
                            ps = psp.tile([P, CH], F32, name=f"{pfx}_ps2", tag="ps")
                            for k in range(HT):
                                nc.tensor.matmul(
                                    ps[:], _r(ws[:, k, :]),
                                    _r(h1[:, k, ch * CH:(ch + 1) * CH]),
                                    start=(k == 0), stop=(k == HT - 1))
                            nc.scalar.activation(
                                h2[:, m, ch * CH:(ch + 1) * CH].bitcast(F32R), ps[:],
                                                 AF.Relu, bias=b2[:, m:m + 1])
                    for m in range(HT):
                        ws = wstp.tile([P, HT, P], F32, name=f"{pfx}_w3", tag="wst")
                        nc.sync.dma_start(
                            ws[:].bitcast(F32R),
                            W3[:, m * P:(m + 1) * P]
                            .rearrange("(k p) m -> p k m", p=P).bitcast(F32R))
                        for ch in range(CHN):
                            ps = psp.tile([P, CH], F32, name=f"{pfx}_ps3", tag="ps")
                            for k in range(HT):
                                nc.tensor.matmul(
                                    ps[:], _r(ws[:, k, :]),
                                    _r(h2[:, k, ch * CH:(ch + 1) * CH]),
                                    start=(k == 0), stop=(k == HT - 1))
                            consume(m, ch, ps, b3)

                # ---------------- phase F: f-MLP on x1 shard and x2 shard --------
                for a, src in (((0, xaT), (1, xbT)) if on("F") else ()):
                    def f_consume(m, ch, ps, b3, a=a):
                        ev = evp.tile([P, CH], F32, name="f_ev", tag="ev")
                        nc.scalar.activation(ev[:], ps[:], AF.Relu, bias=b3[:, m:m + 1])
                        nc.sync.dma_start(
                            ag_in[a, m * P:(m + 1) * P, ch * CH:(ch + 1) * CH], ev[:])
                    mlp3(src.rearrange("(k p) t -> p k t", p=P), ET,
                         w_in["fW1"], w_in["fW2"], w_in["fW3"],
                         btiles["fb1"], btiles["fb2"], btiles["fb3"], f_consume,
                         f"F{a}")

                # ---------------- AllGather faT/fbT ----------------
                if on("AG"):
                    cc("AllGather", ALU.bypass, groups, ag_in, ag_out)

                # ---------------- g-MLP stream machinery ----------------
                vsum = smlp.tile([P, HT, 4], F32, tag="vsum")

                def g_stream(s, src_ap, pfx):
                    vred = smlp.tile([P, HT, CHN], F32, name=f"{pfx}_vred", tag="vred")

                    def g_consume(m, ch, ps, b3):
                        ev = evp.tile([P, CH], F32, name="g_ev", tag="ev")
                        nc.scalar.activation(ev[:], ps[:], AF.Relu, bias=b3[:, m:m + 1])
                        nc.vector.tensor_reduce(vred[:, m, ch:ch + 1], ev[:],
                                                axis=mybir.AxisListType.X, op=ALU.add)
                    mlp3(src_ap, ET, w_in["gW1"], w_in["gW2"], w_in["gW3"],
                         btiles["gb1"], btiles["gb2"], btiles["gb3"], g_consume, pfx)
                    nc.vector.tensor_reduce(vsum[:, :, s:s + 1], vred[:],
                                            axis=mybir.AxisListType.X, op=ALU.add)

                # g on x1 shard: overlaps the AllGather
                if on("Gxa"):
                    g_stream(0, xaT.rearrange("(k p) t -> p k t", p=P), "Gxa")

                # ---------------- phase S: attention scores, exp, partials -------
                # S_A: tA = exp(fa_full @ fbT_r)   [all i (part-tiles), my j (free)]
                # S_B: tB = exp(fb_full @ faT_r)   [all j (part-tiles), my i (free)]
                rsA = smlp.tile([P, IT], F32, tag="rsA")   # partial row sums (over my j)
                rsB = smlp.tile([P, IT], F32, tag="rsB")   # partial col sums (over my i)
                if on("S"):
                    rhsA = bigp.tile([P, HT, Ls], F32, name="rhsA", tag="bigA")
                    dma_split(rhsA[:].bitcast(F32R),
                              ag_in[1].rearrange("(k p) t -> p k t", p=P)
                              .bitcast(F32R), 4)
                    rhsB = bigp.tile([P, HT, Ls], F32, name="rhsB", tag="bigB")
                    dma_split(rhsB[:].bitcast(F32R),
                              ag_in[0].rearrange("(k p) t -> p k t", p=P)
                              .bitcast(F32R), 4)
                for a, (rhs_t, tdst, rsum) in (
                        enumerate(((rhsA, tA, rsA), (rhsB, tB, rsB)))
                        if on("S") else ()):
                    for im in range(IT):
                        ws = wstp.tile([P, HT, P], F32, name="s_lhs", tag="wst")
                        nc.sync.dma_start(
                            ws[:].bitcast(F32R),
                            ag_out[im // (IT // G), a, :,
                                   (im % (IT // G)) * P:(im % (IT // G) + 1) * P]
                            .rearrange("(k p) i -> p k i", p=P).bitcast(F32R))
                        et = rowp.tile([P, Ls], F32, name="s_exp", tag="row",
                                       bufs=1)
                        for jc in range(CHN):
                            ps = psp.tile([P, CH], F32, name="s_ps", tag="ps")
                            for k in range(HT):
                                nc.tensor.matmul(
                                    ps[:], _r(ws[:, k, :]),
                                    _r(rhs_t[:, k, jc * CH:(jc + 1) * CH]),
                                    start=(k == 0), stop=(k == HT - 1))
                            nc.scalar.activation(et[:, jc * CH:(jc + 1) * CH], ps[:],
                                                 AF.Exp)
                        nc.vector.tensor_reduce(rsum[:, im:im + 1], et[:],
                                                axis=mybir.AxisListType.X, op=ALU.add)
                        nc.sync.dma_start(tdst[im], et[:])

                # ---------------- ReduceScatter row/col sums ----------------
                # rc_in[s, 0, :] = r partials for i-shard s; [s, 1, :] = c partials.
                mloc = Ls // P
                for s in (range(G) if on("RC") else ()):
                    nc.sync.dma_start(
                        rc_in[s, 0, :].rearrange("(m p) -> p m", p=P),
                        rsA[:, s * mloc:(s + 1) * mloc])
                    nc.sync.dma_start(
                        rc_in[s, 1, :].rearrange("(m p) -> p m", p=P),
                        rsB[:, s * mloc:(s + 1) * mloc])
                if on("RC"):
                    cc("ReduceScatter", ALU.add, groups, rc_in, rc_out)
                # broadcast + reciprocal -> [P, Ls] scale rows (one slot, reused)
                def make_inv(which, nm):
                    dst = smlp.tile([P, Ls], F32, name=nm, tag="rcinv", bufs=1)
                    t1 = rowp.tile([1, Ls], F32, name="rc_row", tag="row",
                                   bufs=1)
                    nc.sync.dma_start(t1[:], rc_out[which][None, :])
                    bc = wstp.tile([P, Ls], F32, name="rc_bc", tag="wst")
                    nc.gpsimd.partition_broadcast(bc[:], t1[:])
                    nc.vector.reciprocal(dst[:], bc[:])
                    return dst

                rinv = make_inv(0, "rinv") if on("BA") else None

                # ---------------- beta / alpha contractions ----------------
                # betaT[e, i_my] = sum_j x2[j, e] * tB[j, i_my]  (then * rinv)
                # alphaT[e, j_my] = sum_i x1[i, e] * tA[i, j_my] (then * cinv)
                if on("Gxb"):
                    g_stream(1, xbT.rearrange("(k p) t -> p k t", p=P), "Gxb")
                cinv = make_inv(1, "cinv") if on("BA") else None
                for xsrc, tsrc, scl, spill, pfx in ((
                        (x2f, tB, rinv, sp_beta, "bt"),
                        (x1f, tA, cinv, sp_alpha, "al")) if on("BA") else ()):
                    xlo = bigp.tile([P, IT // 2, E], F32, name=f"{pfx}_xlo", tag="bigA")
                    dma_split(
                        xlo[:].bitcast(F32R),
                        xsrc[:L // 2].rearrange("(k p) e -> p k e", p=P)
                        .bitcast(F32R), 8)
                    xhi = bigp.tile([P, IT // 2, E], F32, name=f"{pfx}_xhi", tag="bigB")
                    dma_split(
                        xhi[:].bitcast(F32R),
                        xsrc[L // 2:].rearrange("(k p) e -> p k e", p=P)
                        .bitcast(F32R), 8)
                    RTB = 1  # j_k rows per rt DMA
                    for ch in range(CHN):
                        pss = [psp.tile([P, CH], F32, name=f"{pfx}_ps{e}", tag="ps")
                               for e in range(ET)]
                        for jk0 in range(0, IT, RTB):
                            rt = rowp.tile([P, RTB, CH], F32, name=f"{pfx}_rt",
                                           tag="rt", bufs=8)
                            nc.sync.dma_start(
                                rt[:],
                                tsrc[jk0:jk0 + RTB, :, ch * CH:(ch + 1) * CH]
                                .rearrange("k p t -> p k t"))
                            # fold the softmax 1/denominator into the T rows
                            nc.vector.tensor_tensor(
                                rt[:].bitcast(F32R), rt[:],
                                scl[:, None, ch * CH:(ch + 1) * CH]
                                .to_broadcast([P, RTB, CH]), ALU.mult)
                            for jo in range(RTB):
                                jk = jk0 + jo
                                xt = xlo if jk < IT // 2 else xhi
                                jl = jk % (IT // 2)
                                for e in range(ET):
                                    nc.tensor.matmul(
                                        pss[e][:],
                                        _r(xt[:, jl, e * P:(e + 1) * P]),
                                        _r(rt[:, jo, :]),
                                        start=(jk == 0), stop=(jk == IT - 1))
                        for e in range(ET):
                            ev = evp.tile([P, CH], F32, name=f"{pfx}_ev", tag="ev")
                            nc.scalar.copy(ev[:], pss[e][:])
                            nc.sync.dma_start(
                                spill[e, :, ch * CH:(ch + 1) * CH], ev[:])

                # ---------------- remaining g-MLP streams ----------------
                if on("Gbt"):
                    g_stream(2, sp_beta.rearrange("m p t -> p m t"), "Gbt")
                if on("Gal"):
                    g_stream(3, sp_alpha.rearrange("m p t -> p m t"), "Gal")

                if on("H"):
                    # ---------------- aggregate + h-MLP ----------------
                    # v1 = g(x1).sum + g(beta).sum ; v2 = g(x2).sum + g(alpha).sum
                    v12 = smlp.tile([P, HT, 2], F32, tag="v12")
                    nc.vector.tensor_tensor(v12[:, :, 0:1], vsum[:, :, 0:1],
                                            vsum[:, :, 2:3], ALU.add)
                    nc.vector.tensor_tensor(v12[:, :, 1:2], vsum[:, :, 1:2],
                                            vsum[:, :, 3:4], ALU.add)
                    nc.sync.dma_start(vs_in[:H].rearrange("(m p) -> p m", p=P),
                                      v12[:, :, 0])
                    nc.sync.dma_start(vs_in[H:].rearrange("(m p) -> p m", p=P),
                                      v12[:, :, 1])
                    cc("ReduceScatter", ALU.add, groups, vs_in, vs_out)
                    aggT = smlp.tile([P, KS1], F32, tag="aggT")
                    nc.sync.dma_start(aggT[:], vs_out.rearrange("(m p) -> p m", p=P))

                    # h layer 1 (k-split partial -> ReduceScatter -> bias+relu)
                    h1p = smlp.tile([P, HT], F32, tag="h1p")
                    for m in range(HT):
                        ws = wstp.tile([P, KS1, P], F32, name="h1_w", tag="wst")
                        nc.sync.dma_start(
                            ws[:], hW1s[:, m * P:(m + 1) * P]
                            .rearrange("(k p) m -> p k m", p=P))
                        ps = psp.tile([P, CH], F32, name="h1_ps", tag="ps")
                        for k in range(KS1):
                            nc.tensor.matmul(ps[:, 0:1], ws[:, k, :],
                                             aggT[:, k:k + 1],
                                             start=(k == 0), stop=(k == KS1 - 1))
                        nc.scalar.copy(h1p[:, m:m + 1], ps[:, 0:1])
                    nc.sync.dma_start(h1_in.rearrange("(m p) -> p m", p=P), h1p[:])
                    cc("ReduceScatter", ALU.add, groups, h1_in, h1_out)
                    h1s = smlp.tile([P, KS2], F32, tag="h1s")
                    nc.sync.dma_start(h1s[:], h1_out.rearrange("(m p) -> p m", p=P))
                    nc.vector.tensor_tensor(h1s[:], h1s[:], hb1s_t[:], ALU.add)
                    h1sr = smlp.tile([P, KS2], F32, tag="h1sr")
                    nc.scalar.activation(h1sr[:], h1s[:], AF.Relu)

                    # h layer 2 (k-split partial -> AllReduce -> bias+relu)
                    h2p = smlp.tile([P, HT], F32, tag="h2p")
                    for m in range(HT):
                        ws = wstp.tile([P, KS2, P], F32, name="h2_w", tag="wst")
                        nc.sync.dma_start(
                            ws[:], hW2s[:, m * P:(m + 1) * P]
                            .rearrange("(k p) m -> p k m", p=P))
                        ps = psp.tile([P, CH], F32, name="h2_ps", tag="ps")
                        for k in range(KS2):
                            nc.tensor.matmul(ps[:, 0:1], ws[:, k, :],
                                             h1sr[:, k:k + 1],
                                             start=(k == 0), stop=(k == KS2 - 1))
                        nc.scalar.copy(h2p[:, m:m + 1], ps[:, 0:1])
                    nc.sync.dma_start(h2_in.rearrange("(m p) -> p m", p=P), h2p[:])
                    cc("AllReduce", ALU.add, groups, h2_in, h2_out)
                    h2s = smlp.tile([P, HT], F32, tag="h2s")
                    nc.sync.dma_start(h2s[:], h2_out.rearrange("(m p) -> p m", p=P))
                    nc.vector.tensor_tensor(h2s[:], h2s[:], hb2_t[:], ALU.add)
                    h2sr = smlp.tile([P, HT], F32, tag="h2sr")
                    nc.scalar.activation(h2sr[:], h2s[:], AF.Relu)

                    # h layer 3 (full, every core; O x 1 output)
                    w3t = smlp.tile([P, HT, O], F32, tag="w3t")
                    nc.sync.dma_start(w3t[:], hW3.rearrange("(k p) o -> p k o", p=P))
                    ps = psp.tile([P, CH], F32, name="h3_ps", tag="ps")
                    for k in range(HT):
                        nc.tensor.matmul(ps[:O, 0:1], w3t[:, k, :], h2sr[:, k:k + 1],
                                         start=(k == 0), stop=(k == HT - 1))
                    ot = smlp.tile([O, 1], F32, tag="ot")
                    nc.scalar.activation(ot[:], ps[:O, 0:1], AF.Relu, bias=hb3_t[:])
                    nc.sync.dma_start(out[:], ot[:])

    nc.compile()
    return nc


def make_in_maps(inputs, B=2, L=4096, E=1024, H=2048, O=3, n_cores=8):
    G = n_cores // B
    Ls = L // G
    shared = {}
    for nm in ("fW1", "fW2", "fW3", "gW1", "gW2", "gW3",
               "fb1", "fb2", "fb3", "gb1", "gb2", "gb3", "hW3", "hb2", "hb3"):
        shared[nm] = np.ascontiguousarray(np.asarray(inputs[nm], dtype=np.float32))
    hW1 = np.asarray(inputs["hW1"], dtype=np.float32)
    hW2 = np.asarray(inputs["hW2"], dtype=np.float32)
    hb1 = np.asarray(inputs["hb1"], dtype=np.float32)
    x1 = np.asarray(inputs["x1"], dtype=np.float32)
    x2 = np.asarray(inputs["x2"], dtype=np.float32)
    in_maps = []
    for c in range(n_cores):
        g, r = c // G, c % G
        m = dict(shared)
        m["xaT"] = np.ascontiguousarray(x1[g, r * Ls:(r + 1) * Ls, :].T)
        m["xbT"] = np.ascontiguousarray(x2[g, r * Ls:(r + 1) * Ls, :].T)
        m["x1f"] = np.ascontiguousarray(x1[g])
        m["x2f"] = np.ascontiguousarray(x2[g])
        k1 = 2 * H // G
        m["hW1s"] = np.ascontiguousarray(hW1[r * k1:(r + 1) * k1, :])
        k2 = H // G
        m["hW2s"] = np.ascontiguousarray(hW2[r * k2:(r + 1) * k2, :])
        m["hb1s"] = np.ascontiguousarray(hb1[r * k2:(r + 1) * k2])
        in_maps.append(m)
    return in_maps


def assemble_out(results, B=2, n_cores=8):
    G = n_cores // B
    return np.stack([results[g * G]["out"][:, 0] for g in range(B)]).astype(
        np.float32)


_NC_CACHE = {}


def kernel(**inputs):
    B, L, E = inputs["x1"].shape
    H = inputs["fW1"].shape[1]
    O = inputs["hW3"].shape[1]
    n_cores = 8
    key = (B, L, E, H, O, n_cores)
    if key not in _NC_CACHE:
        _NC_CACHE[key] = build_nc(B, L, E, H, O, n_cores)
    nc = _NC_CACHE[key]
    in_maps = make_in_maps(inputs, B, L, E, H, O, n_cores)
    res = bass_utils.run_bass_kernel_spmd(nc, in_maps,
                                          core_ids=list(range(n_cores)))
    return assemble_out(res.results, B, n_cores)



# revision 15
# speedup vs baseline: 1.2689x; 1.2689x over previous
"""Trainium2 Bass kernel for DecomposableAttention (B=2, L=4096, E=1024, H=2048, O=3).

Sharding: 8 cores = 2 groups of 4 (one per batch element). Within a group the
sequence dim L is sharded 4 ways (Ls=1024 rows per core). Cross-core data:
  - two AllGathers (faT then fbT, bf16) pipelined under f-MLP / g-MLP compute
  - two ReduceScatters of exp-row/col-sum partials (softmax denominators)
  - ReduceScatter / AllReduce for the tiny aggregate h-MLP.
Both attention orientations exp(fa@fb^T) and exp(fb@fa^T) are computed locally
([all x shard] each) so that beta and alpha contractions are fully local.

All big matmuls run in bf16 (fp32 PSUM accumulation); weights are converted to
bf16 on the host. The softmax 1/denominator is folded into the PSUM->SBUF
evacuation of the beta/alpha contraction outputs.
"""

import numpy as np

try:
    import concourse.bass as bass
except ImportError:  # fall back to the staged repo checkout
    import sys
    for p in ("/opt/trn_rl_repo", "/root/.axon_site/_ro/trn_rl_repo"):
        if p not in sys.path:
            sys.path.insert(0, p)
    import concourse.bass as bass
import concourse.mybir as mybir
import concourse.tile as tile
from concourse import bacc
from concourse import bass_utils

F32 = mybir.dt.float32
F32R = mybir.dt.float32r
BF16 = mybir.dt.bfloat16


def _r(ap):
    return ap.bitcast(F32R)
AF = mybir.ActivationFunctionType
ALU = mybir.AluOpType
P = 128
CH = 512  # moving free-dim chunk (1 fp32 PSUM bank)


def build_nc(B=2, L=4096, E=1024, H=2048, O=3, n_cores=8, reps=1,
             mock_collectives=False, phases=None):
    """Build the SPMD Bass program (identical on all cores; per-core inputs)."""
    G = n_cores // B          # cores per batch group
    Ls = L // G               # sequence shard per core
    ET, HT, IT = E // P, H // P, L // P       # 128-tiles per dim
    CHN = Ls // CH            # free chunks per token block
    KT1 = 2 * H // P          # agg dim tiles (h layer 1)
    KS1 = KT1 // G            # per-core k-tiles for hW1
    KS2 = (H // P) // G       # per-core k-tiles for hW2
    NH = H // CH              # 512-chunks of H (h-MLP row outputs)
    assert Ls % CH == 0 and E % P == 0 and H % P == 0 and (2 * H) % (G * P) == 0
    assert (H // P) % G == 0

    groups = [list(range(g * G, (g + 1) * G)) for g in range(B)]

    nc = bacc.Bacc("TRN2", target_bir_lowering=False, debug=False,
                   num_devices=n_cores)

    # ---------------- external I/O ----------------
    xaT = nc.dram_tensor("xaT", [E, Ls], BF16, kind="ExternalInput")
    xbT = nc.dram_tensor("xbT", [E, Ls], BF16, kind="ExternalInput")
    x1f = nc.dram_tensor("x1f", [L, E], BF16, kind="ExternalInput")
    x2f = nc.dram_tensor("x2f", [L, E], BF16, kind="ExternalInput")
    w_in = {}
    for nm in ("f", "g"):
        w_in[nm + "W1"] = nc.dram_tensor(nm + "W1", [E, H], BF16, kind="ExternalInput")
        w_in[nm + "W2"] = nc.dram_tensor(nm + "W2", [H, H], BF16, kind="ExternalInput")
        w_in[nm + "W3"] = nc.dram_tensor(nm + "W3", [H, H], BF16, kind="ExternalInput")
        for i in (1, 2, 3):
            w_in[f"{nm}b{i}"] = nc.dram_tensor(f"{nm}b{i}", [H], F32, kind="ExternalInput")
    hW1s = nc.dram_tensor("hW1s", [2 * H // G, H], F32, kind="ExternalInput")
    hW2s = nc.dram_tensor("hW2s", [H // G, H], F32, kind="ExternalInput")
    hW3 = nc.dram_tensor("hW3", [H, O], F32, kind="ExternalInput")
    hb1s = nc.dram_tensor("hb1s", [H // G], F32, kind="ExternalInput")
    hb2 = nc.dram_tensor("hb2", [H], F32, kind="ExternalInput")
    hb3 = nc.dram_tensor("hb3", [O], F32, kind="ExternalInput")
    out = nc.dram_tensor("out", [O, 1], F32, kind="ExternalOutput")

    with tile.TileContext(nc) as tc:
        with (
            tc.tile_pool(name="big", bufs=1) as bigp,
            tc.tile_pool(name="med", bufs=1) as medp,
            tc.tile_pool(name="srhs", bufs=2) as srhsp,
            tc.tile_pool(name="wst", bufs=2) as wstp,
            tc.tile_pool(name="row", bufs=3) as rowp,
            tc.tile_pool(name="ev", bufs=2) as evp,
            tc.tile_pool(name="sml", bufs=1) as smlp,
            tc.tile_pool(name="ps", bufs=8, space="PSUM") as psp,
            tc.tile_pool(name="dram", bufs=1, space="DRAM") as dramp,
        ):
            on = lambda p: phases is None or p in phases
            for _rep in range(reps):
                def cc(kind, op, replica_groups, tin, tout):
                    if not mock_collectives:
                        nc.gpsimd.collective_compute(kind, op,
                                                     replica_groups=replica_groups,
                                                     ins=[tin.opt()],
                                                     outs=[tout.opt()])
                        return
                    if kind == "AllGather":
                        for s in range(G):
                            nc.sync.dma_start(tout[s], tin[:])
                    elif kind == "ReduceScatter":
                        if len(tin.shape) > 1 and tin.shape[0] == G:
                            nc.sync.dma_start(tout[:], tin[0])
                        else:
                            nc.sync.dma_start(tout[:], tin[:tout.shape[0]])
                    else:  # AllReduce
                        nc.sync.dma_start(tout[:], tin[:])

                def dma_split(dst_ap, src_ap, n):
                    K = dst_ap.shape[1]
                    step = max(1, (K + n - 1) // n)
                    for s in range(0, K, step):
                        e = min(K, s + step)
                        nc.sync.dma_start(dst_ap[:, s:e], src_ap[:, s:e])

                # ---------------- DRAM scratch ----------------
                ag_a = dramp.tile([H, Ls], BF16)               # my faT (bf16)
                ag_b = dramp.tile([H, Ls], BF16)               # my fbT (bf16)
                ago_a = dramp.tile([G, H, Ls], BF16)           # gathered faT
                ago_b = dramp.tile([G, H, Ls], BF16)           # gathered fbT
                tA = dramp.tile([IT, P, Ls], BF16)             # exp(S)  [all i, my j]
                tB = dramp.tile([IT, P, Ls], BF16)             # exp(S^T)[all j, my i]
                rcr_in = dramp.tile([G, Ls], F32)              # row-sum partials
                rcr_out = dramp.tile([Ls], F32)
                rcc_in = dramp.tile([G, Ls], F32)              # col-sum partials
                rcc_out = dramp.tile([Ls], F32)
                sp_beta = dramp.tile([ET, P, Ls], BF16)        # betaT spill
                sp_alpha = dramp.tile([ET, P, Ls], BF16)
                vs_in = dramp.tile([2 * H], F32)               # agg partial
                vs_out = dramp.tile([2 * H // G], F32)         # RS: my agg k-slice
                h1_in = dramp.tile([H], F32)
                h1_out = dramp.tile([H // G], F32)
                h2_in = dramp.tile([H], F32)
                h2_out = dramp.tile([H], F32)

                # ---------------- bias tiles ----------------
                btiles = {}
                for nm in ("fb1", "fb2", "fb3", "gb1", "gb2", "gb3"):
                    t = smlp.tile([P, HT], F32, name=f"bt_{nm}", tag=f"bt_{nm}")
                    nc.sync.dma_start(t[:], w_in[nm].rearrange("(m p) -> p m", p=P))
                    btiles[nm] = t
                hb1s_t = smlp.tile([P, KS2], F32, tag="hb1s_t")
                nc.sync.dma_start(hb1s_t[:], hb1s.rearrange("(m p) -> p m", p=P))
                hb2_t = smlp.tile([P, HT], F32, tag="hb2_t")
                nc.sync.dma_start(hb2_t[:], hb2.rearrange("(m p) -> p m", p=P))
                hb3_t = smlp.tile([O, 1], F32, tag="hb3_t")
                nc.sync.dma_start(hb3_t[:], hb3[:, None])

                # ---------------- helpers ----------------
                def mlp3(src_ap, kt_in, W1, W2, W3, b1, b2, b3, consume, pfx):
                    """3-layer MLP (feature-major activations [P, kt, Ls]), ReLU each
                    layer. src_ap: DRAM AP [P, kt_in, Ls] (bf16). consume(m, ch, psum)
                    is called for the layer-3 output instead of materializing it."""
                    in_t = medp.tile([P, ET, Ls], BF16, name=f"{pfx}_in", tag="inacts")
                    dma_split(in_t[:, :kt_in, :], src_ap, 4)
                    h1 = bigp.tile([P, HT, Ls], BF16, name=f"{pfx}_h1", tag="bigA")
                    for m in range(HT):
                        ws = wstp.tile([P, HT, P], BF16, name=f"{pfx}_w1", tag="wst")
                        nc.sync.dma_start(
                            ws[:, :kt_in, :],
                            W1[:, m * P:(m + 1) * P]
                            .rearrange("(k p) m -> p k m", p=P))
                        for ch in range(CHN):
                            ps = psp.tile([P, CH], F32, name=f"{pfx}_ps1", tag="ps")
                            for k in range(kt_in):
                                nc.tensor.matmul(
                                    ps[:], ws[:, k, :],
                                    in_t[:, k, ch * CH:(ch + 1) * CH],
                                    start=(k == 0), stop=(k == kt_in - 1))
                            nc.scalar.activation(
                                h1[:, m, ch * CH:(ch + 1) * CH], ps[:],
                                AF.Relu, bias=b1[:, m:m + 1])
                    h2 = bigp.tile([P, HT, Ls], BF16, name=f"{pfx}_h2", tag="bigB")
                    for m in range(HT):
                        ws = wstp.tile([P, HT, P], BF16, name=f"{pfx}_w2", tag="wst")
                        nc.sync.dma_start(
                            ws[:],
                            W2[:, m * P:(m + 1) * P]
                            .rearrange("(k p) m -> p k m", p=P))
                        for ch in range(CHN):
                            ps = psp.tile([P, CH], F32, name=f"{pfx}_ps2", tag="ps")
                            for k in range(HT):
                                nc.tensor.matmul(
                                    ps[:], ws[:, k, :],
                                    h1[:, k, ch * CH:(ch + 1) * CH],
                                    start=(k == 0), stop=(k == HT - 1))
                            nc.scalar.activation(
                                h2[:, m, ch * CH:(ch + 1) * CH], ps[:],
                                AF.Relu, bias=b2[:, m:m + 1])
                    for m in range(HT):
                        ws = wstp.tile([P, HT, P], BF16, name=f"{pfx}_w3", tag="wst")
                        nc.sync.dma_start(
                            ws[:],
                            W3[:, m * P:(m + 1) * P]
                            .rearrange("(k p) m -> p k m", p=P))
                        for ch in range(CHN):
                            ps = psp.tile([P, CH], F32, name=f"{pfx}_ps3", tag="ps")
                            for k in range(HT):
                                nc.tensor.matmul(
                                    ps[:], ws[:, k, :],
                                    h2[:, k, ch * CH:(ch + 1) * CH],
                                    start=(k == 0), stop=(k == HT - 1))
                            consume(m, ch, ps, b3)

                # ---------------- phase F0/F1: f-MLP on x1 / x2 shard -----------
                def f_phase(src, ag_dst, pfx):
                    def f_consume(m, ch, ps, b3):
                        ev = evp.tile([P, CH], BF16, name="f_ev", tag="ev")
                        nc.scalar.activation(ev[:], ps[:], AF.Relu, bias=b3[:, m:m + 1])
                        nc.sync.dma_start(
                            ag_dst[m * P:(m + 1) * P, ch * CH:(ch + 1) * CH], ev[:])
                    mlp3(src.rearrange("(k p) t -> p k t", p=P), ET,
                         w_in["fW1"], w_in["fW2"], w_in["fW3"],
                         btiles["fb1"], btiles["fb2"], btiles["fb3"], f_consume, pfx)

                if on("F"):
                    f_phase(xaT, ag_a, "F0")
                if on("AG"):
                    cc("AllGather", ALU.bypass, groups, ag_a, ago_a)
                if on("F"):
                    f_phase(xbT, ag_b, "F1")

                # ---------------- g-MLP stream machinery ----------------
                vsum = smlp.tile([P, HT, 4], F32, tag="vsum")

                def g_stream(s, src_ap, pfx):
                    vred = smlp.tile([P, HT, CHN], F32, name=f"{pfx}_vred", tag="vred")

                    def g_consume(m, ch, ps, b3):
                        ev = evp.tile([P, CH], F32, name="g_ev", tag="ev")
                        nc.scalar.activation(ev[:], ps[:], AF.Relu, bias=b3[:, m:m + 1],
                                             accum_out=vred[:, m, ch:ch + 1])
                    mlp3(src_ap, ET, w_in["gW1"], w_in["gW2"], w_in["gW3"],
                         btiles["gb1"], btiles["gb2"], btiles["gb3"], g_consume, pfx)
                    nc.vector.tensor_reduce(vsum[:, :, s:s + 1], vred[:],
                                            axis=mybir.AxisListType.X, op=ALU.add)

                # g on x1 shard: fills the AllGather windows
                if on("Gxa"):
                    g_stream(0, xaT.rearrange("(k p) t -> p k t", p=P), "Gxa")
                if on("AG"):
                    cc("AllGather", ALU.bypass, groups, ag_b, ago_b)

                # ---------------- phase S: attention scores, exp, partials -------
                # S_A: tA = exp(fa_full @ fbT_r)   [all i (part-tiles), my j (free)]
                # S_B: tB = exp(fb_full @ faT_r)   [all j (part-tiles), my i (free)]
                rsA = smlp.tile([P, IT], F32, tag="rsA")   # partial row sums (over my j)
                rsB = smlp.tile([P, IT], F32, tag="rsB")   # partial col sums (over my i)

                def s_phase(ago_t, rhs_src, rhs_nm, tdst, rsum):
                    rhs_t = srhsp.tile([P, HT, Ls], BF16, name=rhs_nm, tag="srhs")
                    dma_split(rhs_t[:],
                              rhs_src.rearrange("(k p) t -> p k t", p=P), 4)
                    for im in range(IT):
                        ws = wstp.tile([P, HT, P], BF16, name="s_lhs", tag="wst")
                        nc.sync.dma_start(
                            ws[:],
                            ago_t[im // (IT // G), :,
                                  (im % (IT // G)) * P:(im % (IT // G) + 1) * P]
                            .rearrange("(k p) i -> p k i", p=P))
                        et = rowp.tile([P, Ls], BF16, name="s_exp", tag="row",
                                       bufs=2)
                        for jc in range(CHN):
                            ps = psp.tile([P, CH], F32, name="s_ps", tag="ps")
                            for k in range(HT):
                                nc.tensor.matmul(
                                    ps[:], ws[:, k, :],
                                    rhs_t[:, k, jc * CH:(jc + 1) * CH],
                                    start=(k == 0), stop=(k == HT - 1))
                            nc.scalar.activation(et[:, jc * CH:(jc + 1) * CH], ps[:],
                                                 AF.Exp)
                        nc.vector.tensor_reduce(rsum[:, im:im + 1], et[:],
                                                axis=mybir.AxisListType.X, op=ALU.add)
                        nc.sync.dma_start(tdst[im], et[:])

                mloc = Ls // P

                def rc_pack(rsum, rc_in):
                    for s in range(G):
                        nc.sync.dma_start(
                            rc_in[s, :].rearrange("(m p) -> p m", p=P),
                            rsum[:, s * mloc:(s + 1) * mloc])

                if on("S"):
                    s_phase(ago_a, ag_b, "rhsA", tA, rsA)
                    rc_pack(rsA, rcr_in)
                    cc("ReduceScatter", ALU.add, groups, rcr_in, rcr_out)
                    s_phase(ago_b, ag_a, "rhsB", tB, rsB)
                    rc_pack(rsB, rcc_in)
                    cc("ReduceScatter", ALU.add, groups, rcc_in, rcc_out)

                # broadcast + reciprocal -> [P, Ls] scale rows
                def make_inv(rc_out, nm):
                    dst = smlp.tile([P, Ls], BF16, name=nm, tag="rcinv", bufs=2)
                    t1 = rowp.tile([1, Ls], F32, name="rc_row", tag="rcrow",
                                   bufs=2)
                    nc.sync.dma_start(t1[:], rc_out[None, :])
                    bc = wstp.tile([P, Ls], F32, name="rc_bc", tag="wst")
                    nc.gpsimd.partition_broadcast(bc[:], t1[:])
                    with nc.allow_low_precision(
                            reason="softmax 1/denominator applied to bf16 "
                                   "probabilities; 0.4% rel err washes out"):
                        nc.vector.reciprocal(dst[:], bc[:])
                    return dst

                # g on x2 shard: fills the RS/score tail window
                if on("Gxb"):
                    g_stream(1, xbT.rearrange("(k p) t -> p k t", p=P), "Gxb")

                # ---------------- beta / alpha contractions ----------------
                # alphaT[e, j_my] = (sum_i x1[i, e] * tA[i, j_my]) * cinv
                # betaT[e, i_my] = (sum_j x2[j, e] * tB[j, i_my]) * rinv
                cinv = make_inv(rcc_out, "cinv") if on("BA") else None
                rinv = make_inv(rcr_out, "rinv") if on("BA") else None
                for xsrc, tsrc, scl, spill, pfx in ((
                        (x1f, tA, cinv, sp_alpha, "al"),
                        (x2f, tB, rinv, sp_beta, "bt")) if on("BA") else ()):
                    xlo = bigp.tile([P, IT // 2, E], BF16, name=f"{pfx}_xlo", tag="bigA")
                    dma_split(
                        xlo[:],
                        xsrc[:L // 2].rearrange("(k p) e -> p k e", p=P), 8)
                    xhi = bigp.tile([P, IT // 2, E], BF16, name=f"{pfx}_xhi", tag="bigB")
                    dma_split(
                        xhi[:],
                        xsrc[L // 2:].rearrange("(k p) e -> p k e", p=P), 8)
                    for ch in range(CHN):
                        pss = [psp.tile([P, CH], F32, name=f"{pfx}_ps{e}", tag="ps")
                               for e in range(ET)]
                        for jk in range(IT):
                            rt = rowp.tile([P, CH], BF16, name=f"{pfx}_rt",
                                           tag="rt", bufs=8)
                            nc.sync.dma_start(
                                rt[:], tsrc[jk, :, ch * CH:(ch + 1) * CH])
                            xt = xlo if jk < IT // 2 else xhi
                            jl = jk % (IT // 2)
                            for e in range(ET):
                                nc.tensor.matmul(
                                    pss[e][:],
                                    xt[:, jl, e * P:(e + 1) * P],
                                    rt[:],
                                    start=(jk == 0), stop=(jk == IT - 1))
                        for e in range(ET):
                            ev = evp.tile([P, CH], BF16, name=f"{pfx}_ev", tag="ev")
                            nc.vector.tensor_tensor(
                                ev[:], pss[e][:],
                                scl[:, ch * CH:(ch + 1) * CH], ALU.mult)
                            nc.sync.dma_start(
                                spill[e, :, ch * CH:(ch + 1) * CH], ev[:])

                # ---------------- remaining g-MLP streams ----------------
                if on("Gbt"):
                    g_stream(2, sp_beta.rearrange("m p t -> p m t"), "Gbt")
                if on("Gal"):
                    g_stream(3, sp_alpha.rearrange("m p t -> p m t"), "Gal")

                if on("H"):
                    # ---------------- aggregate + h-MLP ----------------
                    # v1 = g(x1).sum + g(beta).sum ; v2 = g(x2).sum + g(alpha).sum
                    v12 = smlp.tile([P, HT, 2], F32, tag="v12")
                    nc.vector.tensor_tensor(v12[:, :, 0:1], vsum[:, :, 0:1],
                                            vsum[:, :, 2:3], ALU.add)
                    nc.vector.tensor_tensor(v12[:, :, 1:2], vsum[:, :, 1:2],
                                            vsum[:, :, 3:4], ALU.add)
                    nc.sync.dma_start(vs_in[:H].rearrange("(m p) -> p m", p=P),
                                      v12[:, :, 0])
                    nc.sync.dma_start(vs_in[H:].rearrange("(m p) -> p m", p=P),
                                      v12[:, :, 1])
                    cc("ReduceScatter", ALU.add, groups, vs_in, vs_out)
                    aggT = smlp.tile([P, KS1], F32, tag="aggT")
                    nc.sync.dma_start(
                        aggT[:].bitcast(F32R),
                        vs_out.rearrange("(m p) -> p m", p=P).bitcast(F32R))

                    # h layer 1: my k-slice of agg x hW1s -> partial h1 row [1, H]
                    # (k-split partial -> ReduceScatter -> bias+relu)
                    ps1 = [psp.tile([P, CH], F32, name=f"h1_ps{n}", tag="ps")
                           for n in range(NH)]
                    for k in range(KS1):
                        wk = wstp.tile([P, H], F32, name="h1_w", tag="wst")
                        nc.sync.dma_start(wk[:].bitcast(F32R),
                                          hW1s[k * P:(k + 1) * P, :].bitcast(F32R))
                        for n in range(NH):
                            nc.tensor.matmul(ps1[n][0:1, :], _r(aggT[:, k:k + 1]),
                                             _r(wk[:, n * CH:(n + 1) * CH]),
                                             start=(k == 0), stop=(k == KS1 - 1))
                    h1row = smlp.tile([1, H], F32, name="h1row", tag="hrow",
                                      bufs=1)
                    for n in range(NH):
                        nc.scalar.copy(h1row[:, n * CH:(n + 1) * CH], ps1[n][0:1, :])
                    nc.sync.dma_start(h1_in[None, :], h1row[:])
                    cc("ReduceScatter", ALU.add, groups, h1_in, h1_out)
                    h1s = smlp.tile([P, KS2], F32, tag="h1s")
                    nc.sync.dma_start(h1s[:], h1_out.rearrange("(m p) -> p m", p=P))
                    nc.vector.tensor_tensor(h1s[:], h1s[:], hb1s_t[:], ALU.add)
                    h1sr = smlp.tile([P, KS2], F32, tag="h1sr")
                    nc.scalar.activation(h1sr[:].bitcast(F32R), h1s[:], AF.Relu)

                    # h layer 2 (k-split partial -> AllReduce -> bias+relu)
                    ps2 = [psp.tile([P, CH], F32, name=f"h2_ps{n}", tag="ps")
                           for n in range(NH)]
                    for k in range(KS2):
                        wk = wstp.tile([P, H], F32, name="h2_w", tag="wst")
                        nc.sync.dma_start(wk[:].bitcast(F32R),
                                          hW2s[k * P:(k + 1) * P, :].bitcast(F32R))
                        for n in range(NH):
                            nc.tensor.matmul(ps2[n][0:1, :], _r(h1sr[:, k:k + 1]),
                                             _r(wk[:, n * CH:(n + 1) * CH]),
                                             start=(k == 0), stop=(k == KS2 - 1))
                    h2row = smlp.tile([1, H], F32, name="h2row", tag="hrow",
                                      bufs=1)
                    for n in range(NH):
                        nc.scalar.copy(h2row[:, n * CH:(n + 1) * CH], ps2[n][0:1, :])
                    nc.sync.dma_start(h2_in[None, :], h2row[:])
                    cc("AllReduce", ALU.add, groups, h2_in, h2_out)
                    h2s = smlp.tile([P, HT], F32, tag="h2s")
                    nc.sync.dma_start(h2s[:], h2_out.rearrange("(m p) -> p m", p=P))
                    nc.vector.tensor_tensor(h2s[:], h2s[:], hb2_t[:], ALU.add)
                    h2sr = smlp.tile([P, HT], F32, tag="h2sr")
                    nc.scalar.activation(h2sr[:], h2s[:], AF.Relu)

                    # h layer 3 (full, every core; O x 1 output)
                    w3t = smlp.tile([P, HT, O], F32, tag="w3t")
                    nc.sync.dma_start(w3t[:], hW3.rearrange("(k p) o -> p k o", p=P))
                    ps = psp.tile([P, CH], F32, name="h3_ps", tag="ps")
                    for k in range(HT):
                        nc.tensor.matmul(ps[:O, 0:1], w3t[:, k, :], h2sr[:, k:k + 1],
                                         start=(k == 0), stop=(k == HT - 1))
                    ot = smlp.tile([O, 1], F32, tag="ot")
                    nc.scalar.activation(ot[:], ps[:O, 0:1], AF.Relu, bias=hb3_t[:])
                    nc.sync.dma_start(out[:], ot[:])

    nc.compile()
    return nc


def make_in_maps(inputs, B=2, L=4096, E=1024, H=2048, O=3, n_cores=8):
    import ml_dtypes
    bf16 = ml_dtypes.bfloat16
    G = n_cores // B
    Ls = L // G
    shared = {}
    for nm in ("fW1", "fW2", "fW3", "gW1", "gW2", "gW3"):
        shared[nm] = np.ascontiguousarray(
            np.asarray(inputs[nm], dtype=np.float32).astype(bf16))
    for nm in ("fb1", "fb2", "fb3", "gb1", "gb2", "gb3", "hW3", "hb2", "hb3"):
        shared[nm] = np.ascontiguousarray(np.asarray(inputs[nm], dtype=np.float32))
    hW1 = np.asarray(inputs["hW1"], dtype=np.float32)
    hW2 = np.asarray(inputs["hW2"], dtype=np.float32)
    hb1 = np.asarray(inputs["hb1"], dtype=np.float32)
    x1 = np.asarray(inputs["x1"], dtype=np.float32)
    x2 = np.asarray(inputs["x2"], dtype=np.float32)
    x1b = x1.astype(bf16)
    x2b = x2.astype(bf16)
    in_maps = []
    for c in range(n_cores):
        g, r = c // G, c % G
        m = dict(shared)
        m["xaT"] = np.ascontiguousarray(x1b[g, r * Ls:(r + 1) * Ls, :].T)
        m["xbT"] = np.ascontiguousarray(x2b[g, r * Ls:(r + 1) * Ls, :].T)
        m["x1f"] = np.ascontiguousarray(x1b[g])
        m["x2f"] = np.ascontiguousarray(x2b[g])
        k1 = 2 * H // G
        m["hW1s"] = np.ascontiguousarray(hW1[r * k1:(r + 1) * k1, :])
        k2 = H // G
        m["hW2s"] = np.ascontiguousarray(hW2[r * k2:(r + 1) * k2, :])
        m["hb1s"] = np.ascontiguousarray(hb1[r * k2:(r + 1) * k2])
        in_maps.append(m)
    return in_maps


def assemble_out(results, B=2, n_cores=8):
    G = n_cores // B
    return np.stack([results[g * G]["out"][:, 0] for g in range(B)]).astype(
        np.float32)


_NC_CACHE = {}


def kernel(**inputs):
    B, L, E = inputs["x1"].shape
    H = inputs["fW1"].shape[1]
    O = inputs["hW3"].shape[1]
    n_cores = 8
    key = (B, L, E, H, O, n_cores)
    if key not in _NC_CACHE:
        _NC_CACHE[key] = build_nc(B, L, E, H, O, n_cores)
    nc = _NC_CACHE[key]
    in_maps = make_in_maps(inputs, B, L, E, H, O, n_cores)
    res = bass_utils.run_bass_kernel_spmd(nc, in_maps,
                                          core_ids=list(range(n_cores)))
    return assemble_out(res.results, B, n_cores)


# revision 17
# speedup vs baseline: 1.2785x; 1.0075x over previous
"""Trainium2 Bass kernel for DecomposableAttention (B=2, L=4096, E=1024, H=2048, O=3).

Sharding: 8 cores = 2 groups of 4 (one per batch element). Within a group the
sequence dim L is sharded 4 ways (Ls=1024 rows per core). Cross-core data:
  - two AllGathers (faT then fbT, bf16) pipelined under f-MLP / g-MLP compute
  - two ReduceScatters of exp-row/col-sum partials (softmax denominators)
  - ReduceScatter / AllReduce for the tiny aggregate h-MLP.
Both attention orientations exp(fa@fb^T) and exp(fb@fa^T) are computed locally
([all x shard] each) so that beta and alpha contractions are fully local.

All big matmuls run in bf16 (fp32 PSUM accumulation); weights are converted to
bf16 on the host. The softmax 1/denominator is folded into the PSUM->SBUF
evacuation of the beta/alpha contraction outputs.
"""

import numpy as np

try:
    import concourse.bass as bass
except ImportError:  # fall back to the staged repo checkout
    import sys
    for p in ("/opt/trn_rl_repo", "/root/.axon_site/_ro/trn_rl_repo"):
        if p not in sys.path:
            sys.path.insert(0, p)
    import concourse.bass as bass
import concourse.mybir as mybir
import concourse.tile as tile
from concourse import bacc
from concourse import bass_utils

F32 = mybir.dt.float32
F32R = mybir.dt.float32r
BF16 = mybir.dt.bfloat16


def _r(ap):
    return ap.bitcast(F32R)
AF = mybir.ActivationFunctionType
ALU = mybir.AluOpType
P = 128
CH = 512  # moving free-dim chunk (1 fp32 PSUM bank)


def build_nc(B=2, L=4096, E=1024, H=2048, O=3, n_cores=8, reps=1,
             mock_collectives=False, phases=None):
    """Build the SPMD Bass program (identical on all cores; per-core inputs)."""
    G = n_cores // B          # cores per batch group
    Ls = L // G               # sequence shard per core
    ET, HT, IT = E // P, H // P, L // P       # 128-tiles per dim
    CHN = Ls // CH            # free chunks per token block
    KT1 = 2 * H // P          # agg dim tiles (h layer 1)
    KS1 = KT1 // G            # per-core k-tiles for hW1
    KS2 = (H // P) // G       # per-core k-tiles for hW2
    NH = H // CH              # 512-chunks of H (h-MLP row outputs)
    assert Ls % CH == 0 and E % P == 0 and H % P == 0 and (2 * H) % (G * P) == 0
    assert (H // P) % G == 0

    groups = [list(range(g * G, (g + 1) * G)) for g in range(B)]

    nc = bacc.Bacc("TRN2", target_bir_lowering=False, debug=False,
                   num_devices=n_cores)

    # ---------------- external I/O ----------------
    xaT = nc.dram_tensor("xaT", [E, Ls], BF16, kind="ExternalInput")
    xbT = nc.dram_tensor("xbT", [E, Ls], BF16, kind="ExternalInput")
    x1f = nc.dram_tensor("x1f", [L, E], BF16, kind="ExternalInput")
    x2f = nc.dram_tensor("x2f", [L, E], BF16, kind="ExternalInput")
    w_in = {}
    for nm in ("f", "g"):
        w_in[nm + "W1"] = nc.dram_tensor(nm + "W1", [E, H], BF16, kind="ExternalInput")
        w_in[nm + "W2"] = nc.dram_tensor(nm + "W2", [H, H], BF16, kind="ExternalInput")
        w_in[nm + "W3"] = nc.dram_tensor(nm + "W3", [H, H], BF16, kind="ExternalInput")
        for i in (1, 2, 3):
            w_in[f"{nm}b{i}"] = nc.dram_tensor(f"{nm}b{i}", [H], F32, kind="ExternalInput")
    hW1s = nc.dram_tensor("hW1s", [2 * H // G, H], F32, kind="ExternalInput")
    hW2s = nc.dram_tensor("hW2s", [H // G, H], F32, kind="ExternalInput")
    hW3 = nc.dram_tensor("hW3", [H, O], F32, kind="ExternalInput")
    hb1s = nc.dram_tensor("hb1s", [H // G], F32, kind="ExternalInput")
    hb2 = nc.dram_tensor("hb2", [H], F32, kind="ExternalInput")
    hb3 = nc.dram_tensor("hb3", [O], F32, kind="ExternalInput")
    out = nc.dram_tensor("out", [O, 1], F32, kind="ExternalOutput")

    with tile.TileContext(nc) as tc:
        with (
            tc.tile_pool(name="big", bufs=1) as bigp,
            tc.tile_pool(name="med", bufs=1) as medp,
            tc.tile_pool(name="srhs", bufs=2) as srhsp,
            tc.tile_pool(name="wst", bufs=4) as wstp,
            tc.tile_pool(name="row", bufs=3) as rowp,
            tc.tile_pool(name="ev", bufs=2) as evp,
            tc.tile_pool(name="sml", bufs=1) as smlp,
            tc.tile_pool(name="ps", bufs=8, space="PSUM") as psp,
            tc.tile_pool(name="dram", bufs=1, space="DRAM") as dramp,
        ):
            on = lambda p: phases is None or p in phases
            for _rep in range(reps):
                def cc(kind, op, replica_groups, tin, tout):
                    if not mock_collectives:
                        nc.gpsimd.collective_compute(kind, op,
                                                     replica_groups=replica_groups,
                                                     ins=[tin.opt()],
                                                     outs=[tout.opt()])
                        return
                    if kind == "AllGather":
                        for s in range(G):
                            nc.sync.dma_start(tout[s], tin[:])
                    elif kind == "ReduceScatter":
                        if len(tin.shape) > 1 and tin.shape[0] == G:
                            nc.sync.dma_start(tout[:], tin[0])
                        else:
                            nc.sync.dma_start(tout[:], tin[:tout.shape[0]])
                    else:  # AllReduce
                        nc.sync.dma_start(tout[:], tin[:])

                def dma_split(dst_ap, src_ap, n, eng=None):
                    eng = eng or nc.gpsimd
                    K = dst_ap.shape[1]
                    step = max(1, (K + n - 1) // n)
                    for s in range(0, K, step):
                        e = min(K, s + step)
                        eng.dma_start(dst_ap[:, s:e], src_ap[:, s:e])

                # ---------------- DRAM scratch ----------------
                ag_a = dramp.tile([H, Ls], BF16)               # my faT (bf16)
                ag_b = dramp.tile([H, Ls], BF16)               # my fbT (bf16)
                ago_a = dramp.tile([G, H, Ls], BF16)           # gathered faT
                ago_b = dramp.tile([G, H, Ls], BF16)           # gathered fbT
                tA = dramp.tile([IT, P, Ls], BF16)             # exp(S)  [all i, my j]
                tB = dramp.tile([IT, P, Ls], BF16)             # exp(S^T)[all j, my i]
                rcr_in = dramp.tile([G, Ls], F32)              # row-sum partials
                rcr_out = dramp.tile([Ls], F32)
                rcc_in = dramp.tile([G, Ls], F32)              # col-sum partials
                rcc_out = dramp.tile([Ls], F32)
                sp_beta = dramp.tile([ET, P, Ls], BF16)        # betaT spill
                sp_alpha = dramp.tile([ET, P, Ls], BF16)
                vs_in = dramp.tile([2 * H], F32)               # agg partial
                vs_out = dramp.tile([2 * H // G], F32)         # RS: my agg k-slice
                h1_in = dramp.tile([H], F32)
                h1_out = dramp.tile([H // G], F32)
                h2_in = dramp.tile([H], F32)
                h2_out = dramp.tile([H], F32)

                # ---------------- bias tiles ----------------
                btiles = {}
                for nm in ("fb1", "fb2", "fb3", "gb1", "gb2", "gb3"):
                    t = smlp.tile([P, HT], F32, name=f"bt_{nm}", tag=f"bt_{nm}")
                    nc.sync.dma_start(t[:], w_in[nm].rearrange("(m p) -> p m", p=P))
                    btiles[nm] = t
                hb1s_t = smlp.tile([P, KS2], F32, tag="hb1s_t")
                nc.sync.dma_start(hb1s_t[:], hb1s.rearrange("(m p) -> p m", p=P))
                hb2_t = smlp.tile([P, HT], F32, tag="hb2_t")
                nc.sync.dma_start(hb2_t[:], hb2.rearrange("(m p) -> p m", p=P))
                hb3_t = smlp.tile([O, 1], F32, tag="hb3_t")
                nc.sync.dma_start(hb3_t[:], hb3[:, None])

                # ---------------- helpers ----------------
                def mlp3(src_ap, kt_in, W1, W2, W3, b1, b2, b3, consume, pfx):
                    """3-layer MLP (feature-major activations [P, kt, Ls]), ReLU each
                    layer. src_ap: DRAM AP [P, kt_in, Ls] (bf16). consume(m, ch, psum)
                    is called for the layer-3 output instead of materializing it."""
                    in_t = medp.tile([P, ET, Ls], BF16, name=f"{pfx}_in", tag="inacts")
                    dma_split(in_t[:, :kt_in, :], src_ap, 4)
                    h1 = bigp.tile([P, HT, Ls], BF16, name=f"{pfx}_h1", tag="bigA")
                    for m in range(HT):
                        ws = wstp.tile([P, HT, P], BF16, name=f"{pfx}_w1", tag="wst")
                        nc.sync.dma_start(
                            ws[:, :kt_in, :],
                            W1[:, m * P:(m + 1) * P]
                            .rearrange("(k p) m -> p k m", p=P))
                        for ch in range(CHN):
                            ps = psp.tile([P, CH], F32, name=f"{pfx}_ps1", tag="ps")
                            for k in range(kt_in):
                                nc.tensor.matmul(
                                    ps[:], ws[:, k, :],
                                    in_t[:, k, ch * CH:(ch + 1) * CH],
                                    start=(k == 0), stop=(k == kt_in - 1))
                            nc.scalar.activation(
                                h1[:, m, ch * CH:(ch + 1) * CH], ps[:],
                                AF.Relu, bias=b1[:, m:m + 1])
                    h2 = bigp.tile([P, HT, Ls], BF16, name=f"{pfx}_h2", tag="bigB")
                    for m in range(HT):
                        ws = wstp.tile([P, HT, P], BF16, name=f"{pfx}_w2", tag="wst")
                        nc.sync.dma_start(
                            ws[:],
                            W2[:, m * P:(m + 1) * P]
                            .rearrange("(k p) m -> p k m", p=P))
                        for ch in range(CHN):
                            ps = psp.tile([P, CH], F32, name=f"{pfx}_ps2", tag="ps")
                            for k in range(HT):
                                nc.tensor.matmul(
                                    ps[:], ws[:, k, :],
                                    h1[:, k, ch * CH:(ch + 1) * CH],
                                    start=(k == 0), stop=(k == HT - 1))
                            nc.scalar.activation(
                                h2[:, m, ch * CH:(ch + 1) * CH], ps[:],
                                AF.Relu, bias=b2[:, m:m + 1])
                    for m in range(HT):
                        ws = wstp.tile([P, HT, P], BF16, name=f"{pfx}_w3", tag="wst")
                        nc.sync.dma_start(
                            ws[:],
                            W3[:, m * P:(m + 1) * P]
                            .rearrange("(k p) m -> p k m", p=P))
                        for ch in range(CHN):
                            ps = psp.tile([P, CH], F32, name=f"{pfx}_ps3", tag="ps")
                            for k in range(HT):
                                nc.tensor.matmul(
                                    ps[:], ws[:, k, :],
                                    h2[:, k, ch * CH:(ch + 1) * CH],
                                    start=(k == 0), stop=(k == HT - 1))
                            consume(m, ch, ps, b3)

                # ---------------- phase F0/F1: f-MLP on x1 / x2 shard -----------
                def f_phase(src, ag_dst, pfx):
                    def f_consume(m, ch, ps, b3):
                        ev = evp.tile([P, CH], BF16, name="f_ev", tag="ev")
                        nc.scalar.activation(ev[:], ps[:], AF.Relu, bias=b3[:, m:m + 1])
                        nc.scalar.dma_start(
                            ag_dst[m * P:(m + 1) * P, ch * CH:(ch + 1) * CH], ev[:])
                    mlp3(src.rearrange("(k p) t -> p k t", p=P), ET,
                         w_in["fW1"], w_in["fW2"], w_in["fW3"],
                         btiles["fb1"], btiles["fb2"], btiles["fb3"], f_consume, pfx)

                if on("F"):
                    f_phase(xaT, ag_a, "F0")
                if on("AG"):
                    cc("AllGather", ALU.bypass, groups, ag_a, ago_a)
                if on("F"):
                    f_phase(xbT, ag_b, "F1")

                # ---------------- g-MLP stream machinery ----------------
                vsum = smlp.tile([P, HT, 4], F32, tag="vsum")

                def g_stream(s, src_ap, pfx):
                    vred = smlp.tile([P, HT, CHN], F32, name=f"{pfx}_vred", tag="vred")

                    def g_consume(m, ch, ps, b3):
                        ev = evp.tile([P, CH], F32, name="g_ev", tag="ev")
                        nc.scalar.activation(ev[:], ps[:], AF.Relu, bias=b3[:, m:m + 1],
                                             accum_out=vred[:, m, ch:ch + 1])
                    mlp3(src_ap, ET, w_in["gW1"], w_in["gW2"], w_in["gW3"],
                         btiles["gb1"], btiles["gb2"], btiles["gb3"], g_consume, pfx)
                    nc.vector.tensor_reduce(vsum[:, :, s:s + 1], vred[:],
                                            axis=mybir.AxisListType.X, op=ALU.add)

                # g on x1 shard: fills the AllGather windows
                if on("Gxa"):
                    g_stream(0, xaT.rearrange("(k p) t -> p k t", p=P), "Gxa")
                if on("AG"):
                    cc("AllGather", ALU.bypass, groups, ag_b, ago_b)

                # ---------------- phase S: attention scores, exp, partials -------
                # S_A: tA = exp(fa_full @ fbT_r)   [all i (part-tiles), my j (free)]
                # S_B: tB = exp(fb_full @ faT_r)   [all j (part-tiles), my i (free)]
                rsA = smlp.tile([P, IT], F32, tag="rsA")   # partial row sums (over my j)
                rsB = smlp.tile([P, IT], F32, tag="rsB")   # partial col sums (over my i)

                def s_phase(ago_t, rhs_src, rhs_nm, tdst, rsum):
                    rhs_t = srhsp.tile([P, HT, Ls], BF16, name=rhs_nm, tag="srhs")
                    dma_split(rhs_t[:],
                              rhs_src.rearrange("(k p) t -> p k t", p=P), 4)
                    for im in range(IT):
                        ws = wstp.tile([P, HT, P], BF16, name="s_lhs", tag="wst")
                        nc.sync.dma_start(
                            ws[:],
                            ago_t[im // (IT // G), :,
                                  (im % (IT // G)) * P:(im % (IT // G) + 1) * P]
                            .rearrange("(k p) i -> p k i", p=P))
                        et = rowp.tile([P, Ls], BF16, name="s_exp", tag="row",
                                       bufs=2)
                        for jc in range(CHN):
                            ps = psp.tile([P, CH], F32, name="s_ps", tag="ps")
                            for k in range(HT):
                                nc.tensor.matmul(
                                    ps[:], ws[:, k, :],
                                    rhs_t[:, k, jc * CH:(jc + 1) * CH],
                                    start=(k == 0), stop=(k == HT - 1))
                            nc.scalar.activation(et[:, jc * CH:(jc + 1) * CH], ps[:],
                                                 AF.Exp)
                        nc.vector.tensor_reduce(rsum[:, im:im + 1], et[:],
                                                axis=mybir.AxisListType.X, op=ALU.add)
                        nc.scalar.dma_start(tdst[im], et[:])

                mloc = Ls // P

                def rc_pack(rsum, rc_in):
                    for s in range(G):
                        nc.sync.dma_start(
                            rc_in[s, :].rearrange("(m p) -> p m", p=P),
                            rsum[:, s * mloc:(s + 1) * mloc])

                if on("S"):
                    s_phase(ago_a, ag_b, "rhsA", tA, rsA)
                    rc_pack(rsA, rcr_in)
                    cc("ReduceScatter", ALU.add, groups, rcr_in, rcr_out)
                    s_phase(ago_b, ag_a, "rhsB", tB, rsB)
                    rc_pack(rsB, rcc_in)
                    cc("ReduceScatter", ALU.add, groups, rcc_in, rcc_out)

                # broadcast + reciprocal -> [P, Ls] scale rows
                def make_inv(rc_out, nm):
                    dst = smlp.tile([P, Ls], BF16, name=nm, tag="rcinv", bufs=2)
                    t1 = rowp.tile([1, Ls], F32, name="rc_row", tag="rcrow",
                                   bufs=2)
                    nc.sync.dma_start(t1[:], rc_out[None, :])
                    bc = wstp.tile([P, Ls], F32, name="rc_bc", tag="rcbc",
                                   bufs=1)
                    nc.gpsimd.partition_broadcast(bc[:], t1[:])
                    with nc.allow_low_precision(
                            reason="softmax 1/denominator applied to bf16 "
                                   "probabilities; 0.4% rel err washes out"):
                        nc.vector.reciprocal(dst[:], bc[:])
                    return dst

                # g on x2 shard: fills the RS/score tail window
                if on("Gxb"):
                    g_stream(1, xbT.rearrange("(k p) t -> p k t", p=P), "Gxb")

                # ---------------- beta / alpha contractions ----------------
                # alphaT[e, j_my] = (sum_i x1[i, e] * tA[i, j_my]) * cinv
                # betaT[e, i_my] = (sum_j x2[j, e] * tB[j, i_my]) * rinv
                cinv = make_inv(rcc_out, "cinv") if on("BA") else None
                rinv = make_inv(rcr_out, "rinv") if on("BA") else None
                for xsrc, tsrc, scl, spill, pfx in ((
                        (x1f, tA, cinv, sp_alpha, "al"),
                        (x2f, tB, rinv, sp_beta, "bt")) if on("BA") else ()):
                    xlo = bigp.tile([P, IT // 2, E], BF16, name=f"{pfx}_xlo", tag="bigA")
                    dma_split(
                        xlo[:],
                        xsrc[:L // 2].rearrange("(k p) e -> p k e", p=P), 8)
                    xhi = bigp.tile([P, IT // 2, E], BF16, name=f"{pfx}_xhi", tag="bigB")
                    dma_split(
                        xhi[:],
                        xsrc[L // 2:].rearrange("(k p) e -> p k e", p=P), 8)
                    for ch in range(CHN):
                        pss = [psp.tile([P, CH], F32, name=f"{pfx}_ps{e}", tag="ps")
                               for e in range(ET)]
                        for jk in range(IT):
                            rt = rowp.tile([P, CH], BF16, name=f"{pfx}_rt",
                                           tag="rt", bufs=8)
                            nc.gpsimd.dma_start(
                                rt[:], tsrc[jk, :, ch * CH:(ch + 1) * CH])
                            xt = xlo if jk < IT // 2 else xhi
                            jl = jk % (IT // 2)
                            for e in range(ET):
                                nc.tensor.matmul(
                                    pss[e][:],
                                    xt[:, jl, e * P:(e + 1) * P],
                                    rt[:],
                                    start=(jk == 0), stop=(jk == IT - 1))
                        for e in range(ET):
                            ev = evp.tile([P, CH], BF16, name=f"{pfx}_ev", tag="ev")
                            nc.vector.tensor_tensor(
                                ev[:], pss[e][:],
                                scl[:, ch * CH:(ch + 1) * CH], ALU.mult)
                            nc.scalar.dma_start(
                                spill[e, :, ch * CH:(ch + 1) * CH], ev[:])

                # ---------------- remaining g-MLP streams ----------------
                if on("Gbt"):
                    g_stream(2, sp_beta.rearrange("m p t -> p m t"), "Gbt")
                if on("Gal"):
                    g_stream(3, sp_alpha.rearrange("m p t -> p m t"), "Gal")

                if on("H"):
                    # ---------------- aggregate + h-MLP ----------------
                    # v1 = g(x1).sum + g(beta).sum ; v2 = g(x2).sum + g(alpha).sum
                    v12 = smlp.tile([P, HT, 2], F32, tag="v12")
                    nc.vector.tensor_tensor(v12[:, :, 0:1], vsum[:, :, 0:1],
                                            vsum[:, :, 2:3], ALU.add)
                    nc.vector.tensor_tensor(v12[:, :, 1:2], vsum[:, :, 1:2],
                                            vsum[:, :, 3:4], ALU.add)
                    nc.sync.dma_start(vs_in[:H].rearrange("(m p) -> p m", p=P),
                                      v12[:, :, 0])
                    nc.sync.dma_start(vs_in[H:].rearrange("(m p) -> p m", p=P),
                                      v12[:, :, 1])
                    cc("ReduceScatter", ALU.add, groups, vs_in, vs_out)
                    aggT = smlp.tile([P, KS1], F32, tag="aggT")
                    nc.sync.dma_start(
                        aggT[:].bitcast(F32R),
                        vs_out.rearrange("(m p) -> p m", p=P).bitcast(F32R))

                    # h layer 1: my k-slice of agg x hW1s -> partial h1 row [1, H]
                    # (k-split partial -> ReduceScatter -> bias+relu)
                    ps1 = [psp.tile([P, CH], F32, name=f"h1_ps{n}", tag="ps")
                           for n in range(NH)]
                    for k in range(KS1):
                        wk = wstp.tile([P, H // 2], F32, name="h1_wa", tag="wst")
                        nc.sync.dma_start(
                            wk[:].bitcast(F32R),
                            hW1s[k * P:(k + 1) * P, :H // 2].bitcast(F32R))
                        wk2 = wstp.tile([P, H // 2], F32, name="h1_wb", tag="wst")
                        nc.sync.dma_start(
                            wk2[:].bitcast(F32R),
                            hW1s[k * P:(k + 1) * P, H // 2:].bitcast(F32R))
                        for n in range(NH):
                            wsel, nn = (wk, n) if n < NH // 2 else (wk2, n - NH // 2)
                            nc.tensor.matmul(ps1[n][0:1, :], _r(aggT[:, k:k + 1]),
                                             _r(wsel[:, nn * CH:(nn + 1) * CH]),
                                             start=(k == 0), stop=(k == KS1 - 1))
                    h1row = smlp.tile([1, H], F32, name="h1row", tag="hrow",
                                      bufs=1)
                    for n in range(NH):
                        nc.scalar.copy(h1row[:, n * CH:(n + 1) * CH], ps1[n][0:1, :])
                    nc.sync.dma_start(h1_in[None, :], h1row[:])
                    cc("ReduceScatter", ALU.add, groups, h1_in, h1_out)
                    h1s = smlp.tile([P, KS2], F32, tag="h1s")
                    nc.sync.dma_start(h1s[:], h1_out.rearrange("(m p) -> p m", p=P))
                    nc.vector.tensor_tensor(h1s[:], h1s[:], hb1s_t[:], ALU.add)
                    h1sr = smlp.tile([P, KS2], F32, tag="h1sr")
                    nc.scalar.activation(h1sr[:].bitcast(F32R), h1s[:], AF.Relu)

                    # h layer 2 (k-split partial -> AllReduce -> bias+relu)
                    ps2 = [psp.tile([P, CH], F32, name=f"h2_ps{n}", tag="ps")
                           for n in range(NH)]
                    for k in range(KS2):
                        wk = wstp.tile([P, H // 2], F32, name="h2_wa", tag="wst")
                        nc.sync.dma_start(
                            wk[:].bitcast(F32R),
                            hW2s[k * P:(k + 1) * P, :H // 2].bitcast(F32R))
                        wk2 = wstp.tile([P, H // 2], F32, name="h2_wb", tag="wst")
                        nc.sync.dma_start(
                            wk2[:].bitcast(F32R),
                            hW2s[k * P:(k + 1) * P, H // 2:].bitcast(F32R))
                        for n in range(NH):
                            wsel, nn = (wk, n) if n < NH // 2 else (wk2, n - NH // 2)
                            nc.tensor.matmul(ps2[n][0:1, :], _r(h1sr[:, k:k + 1]),
                                             _r(wsel[:, nn * CH:(nn + 1) * CH]),
                                             start=(k == 0), stop=(k == KS2 - 1))
                    h2row = smlp.tile([1, H], F32, name="h2row", tag="hrow",
                                      bufs=1)
                    for n in range(NH):
                        nc.scalar.copy(h2row[:, n * CH:(n + 1) * CH], ps2[n][0:1, :])
                    nc.sync.dma_start(h2_in[None, :], h2row[:])
                    cc("AllReduce", ALU.add, groups, h2_in, h2_out)
                    h2s = smlp.tile([P, HT], F32, tag="h2s")
                    nc.sync.dma_start(h2s[:], h2_out.rearrange("(m p) -> p m", p=P))
                    nc.vector.tensor_tensor(h2s[:], h2s[:], hb2_t[:], ALU.add)
                    h2sr = smlp.tile([P, HT], F32, tag="h2sr")
                    nc.scalar.activation(h2sr[:], h2s[:], AF.Relu)

                    # h layer 3 (full, every core; O x 1 output)
                    w3t = smlp.tile([P, HT, O], F32, tag="w3t")
                    nc.sync.dma_start(w3t[:], hW3.rearrange("(k p) o -> p k o", p=P))
                    ps = psp.tile([P, CH], F32, name="h3_ps", tag="ps")
                    for k in range(HT):
                        nc.tensor.matmul(ps[:O, 0:1], w3t[:, k, :], h2sr[:, k:k + 1],
                                         start=(k == 0), stop=(k == HT - 1))
                    ot = smlp.tile([O, 1], F32, tag="ot")
                    nc.scalar.activation(ot[:], ps[:O, 0:1], AF.Relu, bias=hb3_t[:])
                    nc.sync.dma_start(out[:], ot[:])

    nc.compile()
    return nc


def make_in_maps(inputs, B=2, L=4096, E=1024, H=2048, O=3, n_cores=8):
    import ml_dtypes
    bf16 = ml_dtypes.bfloat16
    G = n_cores // B
    Ls = L // G
    shared = {}
    for nm in ("fW1", "fW2", "fW3", "gW1", "gW2", "gW3"):
        shared[nm] = np.ascontiguousarray(
            np.asarray(inputs[nm], dtype=np.float32).astype(bf16))
    for nm in ("fb1", "fb2", "fb3", "gb1", "gb2", "gb3", "hW3", "hb2", "hb3"):
        shared[nm] = np.ascontiguousarray(np.asarray(inputs[nm], dtype=np.float32))
    hW1 = np.asarray(inputs["hW1"], dtype=np.float32)
    hW2 = np.asarray(inputs["hW2"], dtype=np.float32)
    hb1 = np.asarray(inputs["hb1"], dtype=np.float32)
    x1 = np.asarray(inputs["x1"], dtype=np.float32)
    x2 = np.asarray(inputs["x2"], dtype=np.float32)
    x1b = x1.astype(bf16)
    x2b = x2.astype(bf16)
    in_maps = []
    for c in range(n_cores):
        g, r = c // G, c % G
        m = dict(shared)
        m["xaT"] = np.ascontiguousarray(x1b[g, r * Ls:(r + 1) * Ls, :].T)
        m["xbT"] = np.ascontiguousarray(x2b[g, r * Ls:(r + 1) * Ls, :].T)
        m["x1f"] = np.ascontiguousarray(x1b[g])
        m["x2f"] = np.ascontiguousarray(x2b[g])
        k1 = 2 * H // G
        m["hW1s"] = np.ascontiguousarray(hW1[r * k1:(r + 1) * k1, :])
        k2 = H // G
        m["hW2s"] = np.ascontiguousarray(hW2[r * k2:(r + 1) * k2, :])
        m["hb1s"] = np.ascontiguousarray(hb1[r * k2:(r + 1) * k2])
        in_maps.append(m)
    return in_maps


def assemble_out(results, B=2, n_cores=8):
    G = n_cores // B
    return np.stack([results[g * G]["out"][:, 0] for g in range(B)]).astype(
        np.float32)


_NC_CACHE = {}


def kernel(**inputs):
    B, L, E = inputs["x1"].shape
    H = inputs["fW1"].shape[1]
    O = inputs["hW3"].shape[1]
    n_cores = 8
    key = (B, L, E, H, O, n_cores)
    if key not in _NC_CACHE:
        _NC_CACHE[key] = build_nc(B, L, E, H, O, n_cores)
    nc = _NC_CACHE[key]
    in_maps = make_in_maps(inputs, B, L, E, H, O, n_cores)
    res = bass_utils.run_bass_kernel_spmd(nc, in_maps,
                                          core_ids=list(range(n_cores)))
    return assemble_out(res.results, B, n_cores)


# revision 21
# speedup vs baseline: 1.7765x; 1.3895x over previous
"""Trainium2 Bass kernel for DecomposableAttention (B=2, L=4096, E=1024, H=2048, O=3).

Sharding: 8 cores = 2 groups of 4 (one per batch element). Within a group the
sequence dim L is sharded 4 ways (Ls=1024 rows per core). Cross-core data:
  - two AllGathers (faT then fbT, bf16) pipelined under f-MLP / g-MLP compute
  - two ReduceScatters of exp-row/col-sum partials (softmax denominators)
  - ReduceScatter / AllReduce for the tiny aggregate h-MLP.
Both attention orientations exp(fa@fb^T) and exp(fb@fa^T) are computed locally
([all x shard] each) so that beta and alpha contractions are fully local.

All big matmuls run in bf16 (fp32 PSUM accumulation); weights are converted to
bf16 on the host. The softmax 1/denominator is folded into the PSUM->SBUF
evacuation of the beta/alpha contraction outputs.
"""

import numpy as np

try:
    import concourse.bass as bass
except ImportError:  # fall back to the staged repo checkout
    import sys
    for p in ("/opt/trn_rl_repo", "/root/.axon_site/_ro/trn_rl_repo"):
        if p not in sys.path:
            sys.path.insert(0, p)
    import concourse.bass as bass
import concourse.mybir as mybir
import concourse.tile as tile
from concourse import bacc
from concourse import bass_utils

F32 = mybir.dt.float32
F32R = mybir.dt.float32r
BF16 = mybir.dt.bfloat16
FP8 = mybir.dt.float8e4
DR = mybir.MatmulPerfMode.DoubleRow
SW = 512.0   # fp8 weight scale
SA = 16.0    # fp8 activation scale


def _r(ap):
    return ap.bitcast(F32R)
AF = mybir.ActivationFunctionType
ALU = mybir.AluOpType
P = 128
CH = 512  # moving free-dim chunk (1 fp32 PSUM bank)


def build_nc(B=2, L=4096, E=1024, H=2048, O=3, n_cores=8, reps=1,
             mock_collectives=False, phases=None):
    """Build the SPMD Bass program (identical on all cores; per-core inputs)."""
    G = n_cores // B          # cores per batch group
    Ls = L // G               # sequence shard per core
    ET, HT, IT = E // P, H // P, L // P       # 128-tiles per dim
    CHN = Ls // CH            # free chunks per token block
    KT1 = 2 * H // P          # agg dim tiles (h layer 1)
    KS1 = KT1 // G            # per-core k-tiles for hW1
    KS2 = (H // P) // G       # per-core k-tiles for hW2
    NH = H // CH              # 512-chunks of H (h-MLP row outputs)
    assert Ls % CH == 0 and E % P == 0 and H % P == 0 and (2 * H) % (G * P) == 0
    assert (H // P) % G == 0

    groups = [list(range(g * G, (g + 1) * G)) for g in range(B)]

    nc = bacc.Bacc("TRN2", target_bir_lowering=False, debug=False,
                   num_devices=n_cores)

    # ---------------- external I/O ----------------
    xaT = nc.dram_tensor("xaT", [E, Ls], BF16, kind="ExternalInput")
    xbT = nc.dram_tensor("xbT", [E, Ls], BF16, kind="ExternalInput")
    xaT8 = nc.dram_tensor("xaT8", [E, Ls], FP8, kind="ExternalInput")
    xbT8 = nc.dram_tensor("xbT8", [E, Ls], FP8, kind="ExternalInput")
    x1f = nc.dram_tensor("x1f", [L, E], BF16, kind="ExternalInput")
    x2f = nc.dram_tensor("x2f", [L, E], BF16, kind="ExternalInput")
    w_in = {}
    WDT = {"fW1": BF16, "fW2": FP8, "fW3": BF16,
           "gW1": FP8, "gW2": FP8, "gW3": BF16}
    for nm in ("f", "g"):
        for i in (1, 2, 3):
            w = f"{nm}W{i}"
            w_in[w] = nc.dram_tensor(w, [E if i == 1 else H, H], WDT[w],
                                     kind="ExternalInput")
            w_in[f"{nm}b{i}"] = nc.dram_tensor(f"{nm}b{i}", [H], F32, kind="ExternalInput")
    hW1s = nc.dram_tensor("hW1s", [2 * H // G, H], F32, kind="ExternalInput")
    hW2s = nc.dram_tensor("hW2s", [H // G, H], F32, kind="ExternalInput")
    hW3 = nc.dram_tensor("hW3", [H, O], F32, kind="ExternalInput")
    hb1s = nc.dram_tensor("hb1s", [H // G], F32, kind="ExternalInput")
    hb2 = nc.dram_tensor("hb2", [H], F32, kind="ExternalInput")
    hb3 = nc.dram_tensor("hb3", [O], F32, kind="ExternalInput")
    out = nc.dram_tensor("out", [O, 1], F32, kind="ExternalOutput")

    with tile.TileContext(nc) as tc:
        with (
            tc.tile_pool(name="big", bufs=1) as bigp,
            tc.tile_pool(name="med", bufs=1) as medp,
            tc.tile_pool(name="srhs", bufs=2) as srhsp,
            tc.tile_pool(name="wst", bufs=4) as wstp,
            tc.tile_pool(name="row", bufs=3) as rowp,
            tc.tile_pool(name="ev", bufs=2) as evp,
            tc.tile_pool(name="sml", bufs=1) as smlp,
            tc.tile_pool(name="ps", bufs=8, space="PSUM") as psp,
            tc.tile_pool(name="dram", bufs=1, space="DRAM") as dramp,
        ):
            on = lambda p: phases is None or p in phases
            for _rep in range(reps):
                def cc(kind, op, replica_groups, tin, tout):
                    if not mock_collectives:
                        nc.gpsimd.collective_compute(kind, op,
                                                     replica_groups=replica_groups,
                                                     ins=[tin.opt()],
                                                     outs=[tout.opt()])
                        return
                    if kind == "AllGather":
                        for s in range(G):
                            nc.sync.dma_start(tout[s], tin[:])
                    elif kind == "ReduceScatter":
                        if len(tin.shape) > 1 and tin.shape[0] == G:
                            nc.sync.dma_start(tout[:], tin[0])
                        else:
                            nc.sync.dma_start(tout[:], tin[:tout.shape[0]])
                    else:  # AllReduce
                        nc.sync.dma_start(tout[:], tin[:])

                def dma_split(dst_ap, src_ap, n, eng=None):
                    eng = eng or nc.gpsimd
                    K = dst_ap.shape[1]
                    step = max(1, (K + n - 1) // n)
                    for s in range(0, K, step):
                        e = min(K, s + step)
                        eng.dma_start(dst_ap[:, s:e], src_ap[:, s:e])

                # ---------------- DRAM scratch ----------------
                ag_a = dramp.tile([H, Ls], FP8)                # my faT (16*fa, fp8)
                ag_b = dramp.tile([H, Ls], FP8)                # my fbT (16*fb, fp8)
                ago_a = dramp.tile([G, H, Ls], FP8)            # gathered faT
                ago_b = dramp.tile([G, H, Ls], FP8)            # gathered fbT
                tA = dramp.tile([IT, P, Ls], BF16)             # exp(S)  [all i, my j]
                tB = dramp.tile([IT, P, Ls], BF16)             # exp(S^T)[all j, my i]
                rcr_in = dramp.tile([G, Ls], F32)              # row-sum partials
                rcr_out = dramp.tile([Ls], F32)
                rcc_in = dramp.tile([G, Ls], F32)              # col-sum partials
                rcc_out = dramp.tile([Ls], F32)
                sp_beta = dramp.tile([ET, P, Ls], FP8)         # 16*betaT spill
                sp_alpha = dramp.tile([ET, P, Ls], FP8)
                vs_in = dramp.tile([2 * H], F32)               # agg partial
                vs_out = dramp.tile([2 * H // G], F32)         # RS: my agg k-slice
                h1_in = dramp.tile([H], F32)
                h1_out = dramp.tile([H // G], F32)
                h2_in = dramp.tile([H], F32)
                h2_out = dramp.tile([H], F32)

                # ---------------- bias tiles ----------------
                btiles = {}
                for nm in ("fb1", "fb2", "fb3", "gb1", "gb2", "gb3"):
                    t = smlp.tile([P, HT], F32, name=f"bt_{nm}", tag=f"bt_{nm}")
                    nc.sync.dma_start(t[:], w_in[nm].rearrange("(m p) -> p m", p=P))
                    btiles[nm] = t
                hb1s_t = smlp.tile([P, KS2], F32, tag="hb1s_t")
                nc.sync.dma_start(hb1s_t[:], hb1s.rearrange("(m p) -> p m", p=P))
                hb2_t = smlp.tile([P, HT], F32, tag="hb2_t")
                nc.sync.dma_start(hb2_t[:], hb2.rearrange("(m p) -> p m", p=P))
                hb3_t = smlp.tile([O, 1], F32, tag="hb3_t")
                nc.sync.dma_start(hb3_t[:], hb3[:, None])

                # ---------------- helpers ----------------
                def dr_mm(ps, ws, act, kt, first_ok=True):
                    # DoubleRow fp8 matmul chain over kt k-tiles (pairs)
                    for q in range(kt // 2):
                        nc.tensor.matmul(
                            ps, ws[:, 2 * q:2 * q + 2, :], act[:, 2 * q:2 * q + 2, :],
                            start=(q == 0), stop=(q == kt // 2 - 1),
                            perf_mode=DR)

                def any_mm(ps, ws, act, kt, wdt):
                    if wdt == FP8:
                        dr_mm(ps, ws, act, kt)
                    else:
                        for k in range(kt):
                            nc.tensor.matmul(ps, ws[:, k, :], act[:, k, :],
                                             start=(k == 0), stop=(k == kt - 1))

                def mlp3(src_ap, kt_in, Ws, bs, scales, consume, pfx):
                    """3-layer mixed fp8/bf16 MLP. Ws: 3 (dram W, dtype) pairs.
                    scales: activation scales for layers 1 and 2 (layer-3 scale
                    lives in consume). fp8 layers see 16x-scaled inputs and
                    512x weights; bf16 layers see unscaled operands."""
                    wd = [dt for _, dt in Ws]
                    in_t = medp.tile([P, ET, Ls], wd[0], name=f"{pfx}_in",
                                     tag="inacts")
                    dma_split(in_t[:, :kt_in, :], src_ap, 4)
                    h1 = bigp.tile([P, HT, Ls], wd[1], name=f"{pfx}_h1", tag="bigA")
                    for m in range(HT):
                        ws = wstp.tile([P, HT, P], wd[0], name=f"{pfx}_w1", tag="wst")
                        nc.sync.dma_start(
                            ws[:, :kt_in, :],
                            Ws[0][0][:, m * P:(m + 1) * P]
                            .rearrange("(k p) m -> p k m", p=P))
                        for ch in range(CHN):
                            ps = psp.tile([P, CH], F32, name=f"{pfx}_ps1", tag="ps")
                            any_mm(ps[:], ws, in_t[:, :kt_in, ch * CH:(ch + 1) * CH],
                                   kt_in, wd[0])
                            nc.scalar.activation(
                                h1[:, m, ch * CH:(ch + 1) * CH], ps[:],
                                AF.Relu, bias=bs[0][:, m:m + 1], scale=scales[0])
                    h2 = bigp.tile([P, HT, Ls], wd[2], name=f"{pfx}_h2", tag="bigB")
                    for m in range(HT):
                        ws = wstp.tile([P, HT, P], wd[1], name=f"{pfx}_w2", tag="wst")
                        nc.sync.dma_start(
                            ws[:],
                            Ws[1][0][:, m * P:(m + 1) * P]
                            .rearrange("(k p) m -> p k m", p=P))
                        for ch in range(CHN):
                            ps = psp.tile([P, CH], F32, name=f"{pfx}_ps2", tag="ps")
                            any_mm(ps[:], ws, h1[:, :, ch * CH:(ch + 1) * CH],
                                   HT, wd[1])
                            nc.scalar.activation(
                                h2[:, m, ch * CH:(ch + 1) * CH], ps[:],
                                AF.Relu, bias=bs[1][:, m:m + 1], scale=scales[1])
                    for m in range(HT):
                        ws = wstp.tile([P, HT, P], wd[2], name=f"{pfx}_w3", tag="wst")
                        nc.sync.dma_start(
                            ws[:],
                            Ws[2][0][:, m * P:(m + 1) * P]
                            .rearrange("(k p) m -> p k m", p=P))
                        for ch in range(CHN):
                            ps = psp.tile([P, CH], F32, name=f"{pfx}_ps3", tag="ps")
                            any_mm(ps[:], ws, h2[:, :, ch * CH:(ch + 1) * CH],
                                   HT, wd[2])
                            consume(m, ch, ps, bs[2])

                # ---------------- phase F0/F1: f-MLP on x1 / x2 shard -----------
                def f_phase(src, ag_dst, pfx):
                    def f_consume(m, ch, ps, b3):
                        ev = evp.tile([P, CH], FP8, name="f_ev", tag="ev")
                        nc.scalar.activation(ev[:], ps[:], AF.Relu, bias=b3[:, m:m + 1],
                                             scale=SA)
                        nc.scalar.dma_start(
                            ag_dst[m * P:(m + 1) * P, ch * CH:(ch + 1) * CH], ev[:])
                    mlp3(src.rearrange("(k p) t -> p k t", p=P), ET,
                         [(w_in["fW1"], BF16), (w_in["fW2"], FP8),
                          (w_in["fW3"], BF16)],
                         [btiles["fb1"], btiles["fb2"], btiles["fb3"]],
                         [SA, 1.0 / (SW * SA)], f_consume, pfx)

                if on("F"):
                    f_phase(xaT, ag_a, "F0")
                if on("AG"):
                    cc("AllGather", ALU.bypass, groups, ag_a, ago_a)
                if on("F"):
                    f_phase(xbT, ag_b, "F1")

                # ---------------- g-MLP stream machinery ----------------
                vsum = smlp.tile([P, HT, 4], F32, tag="vsum")

                def g_stream(s, src_ap, pfx):
                    vred = smlp.tile([P, HT, CHN], F32, name=f"{pfx}_vred", tag="vred")

                    def g_consume(m, ch, ps, b3):
                        ev = evp.tile([P, CH], F32, name="g_ev", tag="ev")
                        nc.scalar.activation(ev[:], ps[:], AF.Relu, bias=b3[:, m:m + 1],
                                             accum_out=vred[:, m, ch:ch + 1])
                    mlp3(src_ap, ET,
                         [(w_in["gW1"], FP8), (w_in["gW2"], FP8),
                          (w_in["gW3"], BF16)],
                         [btiles["gb1"], btiles["gb2"], btiles["gb3"]],
                         [1.0 / SW, 1.0 / (SW * SA)], g_consume, pfx)
                    nc.vector.tensor_reduce(vsum[:, :, s:s + 1], vred[:],
                                            axis=mybir.AxisListType.X, op=ALU.add)

                # g on x1 shard: fills the AllGather windows
                if on("Gxa"):
                    g_stream(0, xaT8.rearrange("(k p) t -> p k t", p=P), "Gxa")
                if on("AG"):
                    cc("AllGather", ALU.bypass, groups, ag_b, ago_b)

                # ---------------- phase S: attention scores, exp, partials -------
                # S_A: tA = exp(fa_full @ fbT_r)   [all i (part-tiles), my j (free)]
                # S_B: tB = exp(fb_full @ faT_r)   [all j (part-tiles), my i (free)]
                rsA = smlp.tile([P, IT], F32, tag="rsA")   # partial row sums (over my j)
                rsB = smlp.tile([P, IT], F32, tag="rsB")   # partial col sums (over my i)

                def s_phase(ago_t, rhs_src, rhs_nm, tdst, rsum):
                    rhs_t = srhsp.tile([P, HT, Ls], FP8, name=rhs_nm, tag="srhs")
                    dma_split(rhs_t[:],
                              rhs_src.rearrange("(k p) t -> p k t", p=P), 4)
                    for im in range(IT):
                        ws = wstp.tile([P, HT, P], FP8, name="s_lhs", tag="wst")
                        nc.sync.dma_start(
                            ws[:],
                            ago_t[im // (IT // G), :,
                                  (im % (IT // G)) * P:(im % (IT // G) + 1) * P]
                            .rearrange("(k p) i -> p k i", p=P))
                        et = rowp.tile([P, Ls], BF16, name="s_exp", tag="row",
                                       bufs=2)
                        for jc in range(CHN):
                            ps = psp.tile([P, CH], F32, name="s_ps", tag="ps")
                            dr_mm(ps[:], ws, rhs_t[:, :, jc * CH:(jc + 1) * CH], HT)
                            nc.scalar.activation(et[:, jc * CH:(jc + 1) * CH], ps[:],
                                                 AF.Exp, scale=1.0 / (SA * SA))
                        nc.vector.tensor_reduce(rsum[:, im:im + 1], et[:],
                                                axis=mybir.AxisListType.X, op=ALU.add)
                        nc.scalar.dma_start(tdst[im], et[:])

                mloc = Ls // P

                def rc_pack(rsum, rc_in):
                    for s in range(G):
                        nc.sync.dma_start(
                            rc_in[s, :].rearrange("(m p) -> p m", p=P),
                            rsum[:, s * mloc:(s + 1) * mloc])

                if on("S"):
                    s_phase(ago_a, ag_b, "rhsA", tA, rsA)
                    rc_pack(rsA, rcr_in)
                    cc("ReduceScatter", ALU.add, groups, rcr_in, rcr_out)
                    s_phase(ago_b, ag_a, "rhsB", tB, rsB)
                    rc_pack(rsB, rcc_in)
                    cc("ReduceScatter", ALU.add, groups, rcc_in, rcc_out)

                # broadcast + reciprocal -> [P, Ls] scale rows
                def make_inv(rc_out, nm):
                    dst = smlp.tile([P, Ls], BF16, name=nm, tag="rcinv", bufs=2)
                    t1 = rowp.tile([1, Ls], F32, name="rc_row", tag="rcrow",
                                   bufs=2)
                    nc.sync.dma_start(t1[:], rc_out[None, :])
                    bc = wstp.tile([P, Ls], F32, name="rc_bc", tag="rcbc",
                                   bufs=1)
                    nc.gpsimd.partition_broadcast(bc[:], t1[:])
                    nc.vector.tensor_scalar_mul(bc[:], bc[:], 1.0 / SA)
                    with nc.allow_low_precision(
                            reason="softmax 1/denominator applied to bf16 "
                                   "probabilities; 0.4% rel err washes out"):
                        nc.vector.reciprocal(dst[:], bc[:])
                    return dst

                # g on x2 shard: fills the RS/score tail window
                if on("Gxb"):
                    g_stream(1, xbT8.rearrange("(k p) t -> p k t", p=P), "Gxb")

                # ---------------- beta / alpha contractions ----------------
                # alphaT[e, j_my] = (sum_i x1[i, e] * tA[i, j_my]) * cinv
                # betaT[e, i_my] = (sum_j x2[j, e] * tB[j, i_my]) * rinv
                cinv = make_inv(rcc_out, "cinv") if on("BA") else None
                rinv = make_inv(rcr_out, "rinv") if on("BA") else None
                for xsrc, tsrc, scl, spill, pfx in ((
                        (x1f, tA, cinv, sp_alpha, "al"),
                        (x2f, tB, rinv, sp_beta, "bt")) if on("BA") else ()):
                    xlo = bigp.tile([P, IT // 2, E], BF16, name=f"{pfx}_xlo", tag="bigA")
                    dma_split(
                        xlo[:],
                        xsrc[:L // 2].rearrange("(k p) e -> p k e", p=P), 8)
                    xhi = bigp.tile([P, IT // 2, E], BF16, name=f"{pfx}_xhi", tag="bigB")
                    dma_split(
                        xhi[:],
                        xsrc[L // 2:].rearrange("(k p) e -> p k e", p=P), 8)
                    for ch in range(CHN):
                        pss = [psp.tile([P, CH], F32, name=f"{pfx}_ps{e}", tag="ps")
                               for e in range(ET)]
                        for jk in range(IT):
                            rt = rowp.tile([P, CH], BF16, name=f"{pfx}_rt",
                                           tag="rt", bufs=8)
                            nc.gpsimd.dma_start(
                                rt[:], tsrc[jk, :, ch * CH:(ch + 1) * CH])
                            xt = xlo if jk < IT // 2 else xhi
                            jl = jk % (IT // 2)
                            for e in range(ET):
                                nc.tensor.matmul(
                                    pss[e][:],
                                    xt[:, jl, e * P:(e + 1) * P],
                                    rt[:],
                                    start=(jk == 0), stop=(jk == IT - 1))
                        for e in range(ET):
                            ev = evp.tile([P, CH], FP8, name=f"{pfx}_ev", tag="ev")
                            nc.vector.tensor_tensor(
                                ev[:], pss[e][:],
                                scl[:, ch * CH:(ch + 1) * CH], ALU.mult)
                            nc.scalar.dma_start(
                                spill[e, :, ch * CH:(ch + 1) * CH], ev[:])

                # ---------------- remaining g-MLP streams ----------------
                if on("Gbt"):
                    g_stream(2, sp_beta.rearrange("m p t -> p m t"), "Gbt")
                if on("Gal"):
                    g_stream(3, sp_alpha.rearrange("m p t -> p m t"), "Gal")

                if on("H"):
                    # ---------------- aggregate + h-MLP ----------------
                    # v1 = g(x1).sum + g(beta).sum ; v2 = g(x2).sum + g(alpha).sum
                    v12 = smlp.tile([P, HT, 2], F32, tag="v12")
                    nc.vector.tensor_tensor(v12[:, :, 0:1], vsum[:, :, 0:1],
                                            vsum[:, :, 2:3], ALU.add)
                    nc.vector.tensor_tensor(v12[:, :, 1:2], vsum[:, :, 1:2],
                                            vsum[:, :, 3:4], ALU.add)
                    nc.sync.dma_start(vs_in[:H].rearrange("(m p) -> p m", p=P),
                                      v12[:, :, 0])
                    nc.sync.dma_start(vs_in[H:].rearrange("(m p) -> p m", p=P),
                                      v12[:, :, 1])
                    cc("ReduceScatter", ALU.add, groups, vs_in, vs_out)
                    aggT = smlp.tile([P, KS1], F32, tag="aggT")
                    nc.sync.dma_start(
                        aggT[:].bitcast(F32R),
                        vs_out.rearrange("(m p) -> p m", p=P).bitcast(F32R))

                    # h layer 1: my k-slice of agg x hW1s -> partial h1 row [1, H]
                    # (k-split partial -> ReduceScatter -> bias+relu)
                    ps1 = [psp.tile([P, CH], F32, name=f"h1_ps{n}", tag="ps")
                           for n in range(NH)]
                    for k in range(KS1):
                        wk = wstp.tile([P, H // 2], F32, name="h1_wa", tag="wst")
                        nc.sync.dma_start(
                            wk[:].bitcast(F32R),
                            hW1s[k * P:(k + 1) * P, :H // 2].bitcast(F32R))
                        wk2 = wstp.tile([P, H // 2], F32, name="h1_wb", tag="wst")
                        nc.sync.dma_start(
                            wk2[:].bitcast(F32R),
                            hW1s[k * P:(k + 1) * P, H // 2:].bitcast(F32R))
                        for n in range(NH):
                            wsel, nn = (wk, n) if n < NH // 2 else (wk2, n - NH // 2)
                            nc.tensor.matmul(ps1[n][0:1, :], _r(aggT[:, k:k + 1]),
                                             _r(wsel[:, nn * CH:(nn + 1) * CH]),
                                             start=(k == 0), stop=(k == KS1 - 1))
                    h1row = smlp.tile([1, H], F32, name="h1row", tag="hrow",
                                      bufs=1)
                    for n in range(NH):
                        nc.scalar.copy(h1row[:, n * CH:(n + 1) * CH], ps1[n][0:1, :])
                    nc.sync.dma_start(h1_in[None, :], h1row[:])
                    cc("ReduceScatter", ALU.add, groups, h1_in, h1_out)
                    h1s = smlp.tile([P, KS2], F32, tag="h1s")
                    nc.sync.dma_start(h1s[:], h1_out.rearrange("(m p) -> p m", p=P))
                    nc.vector.tensor_tensor(h1s[:], h1s[:], hb1s_t[:], ALU.add)
                    h1sr = smlp.tile([P, KS2], F32, tag="h1sr")
                    nc.scalar.activation(h1sr[:].bitcast(F32R), h1s[:], AF.Relu)

                    # h layer 2 (k-split partial -> AllReduce -> bias+relu)
                    ps2 = [psp.tile([P, CH], F32, name=f"h2_ps{n}", tag="ps")
                           for n in range(NH)]
                    for k in range(KS2):
                        wk = wstp.tile([P, H // 2], F32, name="h2_wa", tag="wst")
                        nc.sync.dma_start(
                            wk[:].bitcast(F32R),
                            hW2s[k * P:(k + 1) * P, :H // 2].bitcast(F32R))
                        wk2 = wstp.tile([P, H // 2], F32, name="h2_wb", tag="wst")
                        nc.sync.dma_start(
                            wk2[:].bitcast(F32R),
                            hW2s[k * P:(k + 1) * P, H // 2:].bitcast(F32R))
                        for n in range(NH):
                            wsel, nn = (wk, n) if n < NH // 2 else (wk2, n - NH // 2)
                            nc.tensor.matmul(ps2[n][0:1, :], _r(h1sr[:, k:k + 1]),
                                             _r(wsel[:, nn * CH:(nn + 1) * CH]),
                                             start=(k == 0), stop=(k == KS2 - 1))
                    h2row = smlp.tile([1, H], F32, name="h2row", tag="hrow",
                                      bufs=1)
                    for n in range(NH):
                        nc.scalar.copy(h2row[:, n * CH:(n + 1) * CH], ps2[n][0:1, :])
                    nc.sync.dma_start(h2_in[None, :], h2row[:])
                    cc("AllReduce", ALU.add, groups, h2_in, h2_out)
                    h2s = smlp.tile([P, HT], F32, tag="h2s")
                    nc.sync.dma_start(h2s[:], h2_out.rearrange("(m p) -> p m", p=P))
                    nc.vector.tensor_tensor(h2s[:], h2s[:], hb2_t[:], ALU.add)
                    h2sr = smlp.tile([P, HT], F32, tag="h2sr")
                    nc.scalar.activation(h2sr[:], h2s[:], AF.Relu)

                    # h layer 3 (full, every core; O x 1 output)
                    w3t = smlp.tile([P, HT, O], F32, tag="w3t")
                    nc.sync.dma_start(w3t[:], hW3.rearrange("(k p) o -> p k o", p=P))
                    ps = psp.tile([P, CH], F32, name="h3_ps", tag="ps")
                    for k in range(HT):
                        nc.tensor.matmul(ps[:O, 0:1], w3t[:, k, :], h2sr[:, k:k + 1],
                                         start=(k == 0), stop=(k == HT - 1))
                    ot = smlp.tile([O, 1], F32, tag="ot")
                    nc.scalar.activation(ot[:], ps[:O, 0:1], AF.Relu, bias=hb3_t[:])
                    nc.sync.dma_start(out[:], ot[:])

    nc.compile()
    return nc


def make_in_maps(inputs, B=2, L=4096, E=1024, H=2048, O=3, n_cores=8):
    import ml_dtypes
    bf16 = ml_dtypes.bfloat16
    fp8 = ml_dtypes.float8_e4m3
    to8 = lambda a, s: np.clip(np.asarray(a, np.float32) * s,
                               -240.0, 240.0).astype(fp8)
    G = n_cores // B
    Ls = L // G
    shared = {}
    for nm in ("fW2", "gW1", "gW2"):
        shared[nm] = np.ascontiguousarray(to8(inputs[nm], 512.0))
    for nm in ("fW1", "fW3", "gW3"):
        shared[nm] = np.ascontiguousarray(
            np.asarray(inputs[nm], dtype=np.float32).astype(bf16))
    for nm in ("fb1", "fb3", "gb1"):
        shared[nm] = np.ascontiguousarray(
            np.asarray(inputs[nm], dtype=np.float32) * 16.0)
    for nm in ("fb2", "gb2", "gb3", "hW3", "hb2", "hb3"):
        shared[nm] = np.ascontiguousarray(np.asarray(inputs[nm], dtype=np.float32))
    hW1 = np.asarray(inputs["hW1"], dtype=np.float32)
    hW2 = np.asarray(inputs["hW2"], dtype=np.float32)
    hb1 = np.asarray(inputs["hb1"], dtype=np.float32)
    x1 = np.asarray(inputs["x1"], dtype=np.float32)
    x2 = np.asarray(inputs["x2"], dtype=np.float32)
    x1b = x1.astype(bf16)
    x2b = x2.astype(bf16)
    x18 = to8(x1, 16.0)
    x28 = to8(x2, 16.0)
    in_maps = []
    for c in range(n_cores):
        g, r = c // G, c % G
        m = dict(shared)
        m["xaT"] = np.ascontiguousarray(x1b[g, r * Ls:(r + 1) * Ls, :].T)
        m["xbT"] = np.ascontiguousarray(x2b[g, r * Ls:(r + 1) * Ls, :].T)
        m["xaT8"] = np.ascontiguousarray(x18[g, r * Ls:(r + 1) * Ls, :].T)
        m["xbT8"] = np.ascontiguousarray(x28[g, r * Ls:(r + 1) * Ls, :].T)
        m["x1f"] = np.ascontiguousarray(x1b[g])
        m["x2f"] = np.ascontiguousarray(x2b[g])
        k1 = 2 * H // G
        m["hW1s"] = np.ascontiguousarray(hW1[r * k1:(r + 1) * k1, :])
        k2 = H // G
        m["hW2s"] = np.ascontiguousarray(hW2[r * k2:(r + 1) * k2, :])
        m["hb1s"] = np.ascontiguousarray(hb1[r * k2:(r + 1) * k2])
        in_maps.append(m)
    return in_maps


def assemble_out(results, B=2, n_cores=8):
    G = n_cores // B
    return np.stack([results[g * G]["out"][:, 0] for g in range(B)]).astype(
        np.float32)


_NC_CACHE = {}


def kernel(**inputs):
    B, L, E = inputs["x1"].shape
    H = inputs["fW1"].shape[1]
    O = inputs["hW3"].shape[1]
    n_cores = 8
    key = (B, L, E, H, O, n_cores)
    if key not in _NC_CACHE:
        _NC_CACHE[key] = build_nc(B, L, E, H, O, n_cores)
    nc = _NC_CACHE[key]
    in_maps = make_in_maps(inputs, B, L, E, H, O, n_cores)
    res = bass_utils.run_bass_kernel_spmd(nc, in_maps,
                                          core_ids=list(range(n_cores)))
    return assemble_out(res.results, B, n_cores)
